# revision 1
# baseline (speedup 1.0000x reference)
"""Trainium2 Bass kernel for nn_DecorrelateLossClass (segment_reduce / ridge).

Strategy (class-sharded, collective-free):
  * K=128 classes are assigned 16-per-core across 8 cores (snake order by
    descending class count, so per-slot padded sizes match across cores).
    Classes are processed in PAIRS with a pair-uniform padded width S_p
    (>=128 so the paired Gram's moving dim is >=256, the float32r
    full-rate threshold).
  * The host gathers each core's class columns into a feature-major layout
    xt[128, 4, R] (features chunked 4x128 on partitions, class columns padded
    per-slot on the free dim).
  * Each core computes, per class: per-feature sums/sumsq via bn_stats
    (one instruction per pair+chunk), mean/var/rsqrt stats, normalization
    z=(x-mu)*r written as float32r, the paired sample Gram
    G2 = Z2^T Z2 (contraction over the 512 features on the PE, float32r
    at 1 cycle/row), and Frobenius reductions of the same-class blocks via
    ScalarE Square+accum (cross-class blocks of the pair Gram are ignored).
  * Identity used: sum(corr^2) = ||Xn^T Xn||_F^2 = ||Xn Xn^T||_F^2 (sample
    Gram, ~S x S instead of 512x512), and trace(corr^2) comes analytically
    from the stats.  Zero-padded columns normalize to the constant phantom
    vector -mu*r; their contribution is removed exactly on the host using the
    per-class outputs ||G||^2, P1 (squared last Gram column, guaranteed
    phantom) and rho = ||mu*r||^2.
  * No collectives: the host sums 8x16 per-class scalars.
"""

import os
import sys

import numpy as np

for _p in ("/opt/trn_rl_repo",):
    if os.path.isdir(_p) and _p not in sys.path:
        sys.path.insert(0, _p)

import concourse.bass as bass
from concourse import bacc
import concourse.mybir as mybir
import concourse.tile as tile
from concourse.bass_utils import run_bass_kernel_spmd

K = 128
C = 512
NCH = 4  # feature chunks of 128
NCORES = 8
CLS = 16  # classes per core
NPAIR = CLS // 2
EPS = 1e-8
# fin layout: per class 3 acc cols [0:48], dsq [48:112], murq [112:176],
# per class 3 P1 cols [176:224]
ACC0, DSQ0, MURQ0, P10, OUTW = 0, 48, 112, 176, 224

_nc_cache: dict = {}
_last_results = None


def _legal_pieces(p0, p1):
    """Split [p0,p1) into pieces legal for SBUF partition windows:
    start 0 -> <=128, start 32 -> <=32, start 64 -> <=64, start 96 -> <=32."""
    out = []
    while p0 < p1:
        if p0 == 0:
            end = p1
        elif p0 == 32:
            end = min(p1, 64)
        elif p0 == 64:
            end = min(p1, 128)
        elif p0 == 96:
            end = min(p1, 128)
        else:
            raise AssertionError(f"illegal partition start {p0}")
        out.append((p0, end))
        p0 = end
    return out


def _row_splits(om, m, S_p):
    """Split pair-Gram row chunk [om, om+m) into (class_half, p0, p1) pieces."""
    out = []
    for h, lo, hi in ((0, 0, S_p), (1, S_p, 2 * S_p)):
        a = max(om, lo)
        b = min(om + m, hi)
        if a < b:
            for q0, q1 in _legal_pieces(a - om, b - om):
                out.append((h, q0, q1))
    return out


def _build_nc(slot_sizes: tuple, R: int):
    f32 = mybir.dt.float32
    f32r = mybir.dt.float32r
    nc = bacc.Bacc("TRN2", target_bir_lowering=False)
    xt_d = nc.dram_tensor("xt", [128, NCH, R], f32, kind="ExternalInput")
    cnt_d = nc.dram_tensor("cnt", [128, 5, NCH, CLS], f32, kind="ExternalInput")
    out_d = nc.dram_tensor("outv", [1, OUTW], f32, kind="ExternalOutput")

    pair_w = [slot_sizes[2 * j] for j in range(NPAIR)]  # uniform within pair
    pair_off = [0]
    for j in range(NPAIR):
        assert slot_sizes[2 * j] == slot_sizes[2 * j + 1]
        pair_off.append(pair_off[-1] + 2 * pair_w[j])
    assert pair_off[-1] == R

    AF = mybir.ActivationFunctionType
    OP = mybir.AluOpType
    NG = 2  # pair groups, pipelined
    GPAIR = NPAIR // NG  # pairs per group
    GCLS = CLS // NG  # classes per group

    with tile.TileContext(nc) as tc:
        with (
            tc.tile_pool(name="persist", bufs=1) as persist,
            tc.tile_pool(name="stats", bufs=1) as stats,
            tc.tile_pool(name="scr", bufs=4) as scr,
            tc.tile_pool(name="gram", bufs=6, space="PSUM") as gram,
            tc.tile_pool(name="fpsum", bufs=1, space="PSUM") as fpsum,
        ):
            # ---- persistent tiles (per pair) ----
            x_p = [
                persist.tile(
                    [128, NCH, 2 * pair_w[j]], f32, tag=f"x{j}", name=f"x{j}"
                )
                for j in range(NPAIR)
            ]
            z_p = [
                persist.tile(
                    [128, NCH, 2 * pair_w[j]], f32r, tag=f"z{j}", name=f"z{j}"
                )
                for j in range(NPAIR)
            ]
            cnt_sb = persist.tile([128, 5, NCH, CLS], f32, tag="cnt")
            fin = persist.tile([128, OUTW], f32, tag="fin")
            ones = persist.tile([128, 1], f32, tag="ones")

            nc.vector.memset(fin, 0.0)
            nc.vector.memset(ones, 1.0)

            # ---- DMA in ----
            nc.sync.dma_start(out=cnt_sb, in_=cnt_d[:, :, :, :])
            for j in range(NPAIR):
                o, W2 = pair_off[j], 2 * pair_w[j]
                nc.sync.dma_start(out=x_p[j], in_=xt_d[:, :, o : o + W2])

            V = nc.vector
            murq_view = fin[:, MURQ0 : MURQ0 + 64].rearrange(
                "p (c k) -> p c k", c=NCH
            )
            dsq_view = fin[:, DSQ0 : DSQ0 + 64].rearrange("p (c k) -> p c k", c=NCH)

            for g in range(NG):
                j0 = g * GPAIR  # first pair of group
                k0 = g * GCLS  # first class of group
                bnbuf = stats.tile(
                    [128, NCH, GCLS, 6], f32, tag=f"bnbuf{g}", name=f"bnbuf{g}"
                )
                for jj in range(GPAIR):
                    j = j0 + jj
                    S_p = pair_w[j]
                    for h in range(2):
                        for ch in range(NCH):
                            nc.vector.bn_stats(
                                out=bnbuf[:, ch, 2 * jj + h, :],
                                in_=x_p[j][:, ch, h * S_p : (h + 1) * S_p],
                            )

                # ---- per-group stats math on [128, NCH, GCLS] tiles ----
                def st(tag):
                    return stats.tile(
                        [128, NCH, GCLS], f32, tag=f"{tag}{g}", name=f"{tag}{g}"
                    )

                me = bnbuf[:, :, :, 1]
                ve = bnbuf[:, :, :, 2]
                mo = bnbuf[:, :, :, 4]
                vo = bnbuf[:, :, :, 5]
                gsl = slice(k0, k0 + GCLS)
                nvec = cnt_sb[:, 0, :, gsl]
                rn = cnt_sb[:, 1, :, gsl]
                rn1 = cnt_sb[:, 2, :, gsl]
                cev = cnt_sb[:, 3, :, gsl]
                cov = cnt_sb[:, 4, :, gsl]

                t1 = st("t1")
                t2 = st("t2")
                s1 = st("s1")
                s2 = st("s2")
                e2 = st("e2")
                o2 = st("o2")
                mu = st("mu")
                m2 = st("m2")
                tt = st("tt")
                var = st("var")
                tv = st("tv")
                sq = st("sq")
                r0 = st("r0")
                r2 = st("r2")
                w = st("w")
                r = st("r")
                mur = st("mur")
                nmur = st("nmur")
                d = st("d")

                V.tensor_tensor(out=t1, in0=me, in1=cev, op=OP.mult)  # ce*me
                V.tensor_tensor(out=t2, in0=mo, in1=cov, op=OP.mult)  # co*mo
                V.tensor_tensor(out=s1, in0=t1, in1=t2, op=OP.add)
                V.tensor_tensor(out=e2, in0=t1, in1=me, op=OP.mult)  # ce*me^2
                V.tensor_tensor(out=o2, in0=t2, in1=mo, op=OP.mult)  # co*mo^2
                V.tensor_tensor(out=s2, in0=ve, in1=vo, op=OP.add)
                V.tensor_tensor(out=s2, in0=s2, in1=e2, op=OP.add)
                V.tensor_tensor(out=s2, in0=s2, in1=o2, op=OP.add)  # s2 = sumsq
                V.tensor_tensor(out=mu, in0=s1, in1=rn, op=OP.mult)  # mean
                V.tensor_tensor(out=m2, in0=mu, in1=mu, op=OP.mult)
                V.tensor_tensor(out=m2, in0=m2, in1=nvec, op=OP.mult)  # n*mu^2
                V.tensor_tensor(out=tt, in0=s2, in1=m2, op=OP.subtract)
                V.tensor_tensor(out=var, in0=tt, in1=rn1, op=OP.mult)
                V.tensor_scalar_max(out=var, in0=var, scalar1=0.0)
                V.tensor_scalar_add(out=tv, in0=var, scalar1=float(EPS))
                nc.scalar.sqrt(out=sq, in_=tv)
                V.reciprocal(out=r0, in_=sq)
                # one Newton step: r = r0*(1.5 - 0.5*tv*r0^2)
                V.tensor_tensor(out=r2, in0=r0, in1=r0, op=OP.mult)
                V.tensor_tensor(out=w, in0=tv, in1=r2, op=OP.mult)
                V.tensor_scalar(
                    out=w, in0=w, scalar1=-0.5, scalar2=1.5, op0=OP.mult, op1=OP.add
                )
                V.tensor_tensor(out=r, in0=r0, in1=w, op=OP.mult)
                V.tensor_tensor(out=mur, in0=mu, in1=r, op=OP.mult)
                V.tensor_scalar_mul(out=nmur, in0=mur, scalar1=-1.0)
                V.tensor_tensor(
                    out=murq_view[:, :, gsl], in0=mur, in1=mur, op=OP.mult
                )
                V.tensor_tensor(out=r2, in0=r, in1=r, op=OP.mult)  # r^2
                V.tensor_tensor(out=d, in0=tt, in1=r2, op=OP.mult)  # diag(corr)
                V.tensor_tensor(out=dsq_view[:, :, gsl], in0=d, in1=d, op=OP.mult)

                # ---- per-pair: normalize, Gram, reductions ----
                for jj in range(GPAIR):
                    j = j0 + jj
                    S_p = pair_w[j]
                    W2 = 2 * S_p
                    # normalize both halves: DVE for one class, ACT for other
                    for h in range(2):
                        kk = 2 * jj + h  # class index within group
                        use_act = (kk % 2) == 1
                        for ch in range(NCH):
                            zslice = z_p[j][:, ch, h * S_p : (h + 1) * S_p]
                            xslice = x_p[j][:, ch, h * S_p : (h + 1) * S_p]
                            if use_act:
                                nc.scalar.activation(
                                    out=zslice,
                                    in_=xslice,
                                    func=AF.Identity,
                                    scale=r[:, ch, kk : kk + 1],
                                    bias=nmur[:, ch, kk : kk + 1],
                                )
                            else:
                                V.tensor_scalar(
                                    out=zslice,
                                    in0=xslice,
                                    scalar1=mu[:, ch, kk : kk + 1],
                                    scalar2=r[:, ch, kk : kk + 1],
                                    op0=OP.subtract,
                                    op1=OP.mult,
                                )

                    # paired Gram
                    mchunks = []
                    om = 0
                    while om < W2:
                        mchunks.append((om, min(128, W2 - om)))
                        om += 128
                    nacc = {0: 0, 1: 0}  # per-class-half contribution counter
                    for i, (om, m) in enumerate(mchunks):
                        ps = gram.tile([128, W2], f32, tag="ps", name=f"ps{j}_{i}")
                        for ch in range(NCH):
                            nc.tensor.matmul(
                                ps[:m, :W2],
                                lhsT=z_p[j][:, ch, om : om + m],
                                rhs=z_p[j][:, ch, :W2],
                                start=(ch == 0),
                                stop=(ch == NCH - 1),
                            )
                        # reductions of same-class blocks only
                        for h, p0, p1 in _row_splits(om, m, S_p):
                            k = k0 + 2 * jj + h
                            c0, c1 = h * S_p, (h + 1) * S_p
                            a = nacc[h]
                            nacc[h] += 1
                            assert a < 3
                            sc = scr.tile(
                                [128, 512], f32, tag="scr", name=f"sc{j}_{i}_{h}"
                            )
                            nc.scalar.activation(
                                out=sc[p0:p1, 0:S_p],
                                in_=ps[p0:p1, c0:c1],
                                func=AF.Square,
                                accum_out=fin[
                                    p0:p1, ACC0 + 3 * k + a : ACC0 + 3 * k + a + 1
                                ],
                            )
                            # P1: squared phantom (last) column of this class
                            nc.scalar.activation(
                                out=fin[
                                    p0:p1, P10 + 3 * k + a : P10 + 3 * k + a + 1
                                ],
                                in_=ps[p0:p1, c1 - 1 : c1],
                                func=AF.Square,
                            )

            # ---- final partition reduction via ones-matmul ----
            fps = fpsum.tile([1, OUTW], f32, tag="fps")
            nc.tensor.matmul(fps, lhsT=ones, rhs=fin, start=True, stop=True)
            outsb = persist.tile([1, OUTW], f32, tag="outsb")
            nc.vector.tensor_copy(out=outsb, in_=fps)
            nc.sync.dma_start(out=out_d[:, :], in_=outsb)

    nc.compile()
    return nc


def _ensure_axon_ntff_hook():
    """Register the axon NTFF profiling hook if the image's antenv lacks it."""
    try:
        import types

        import antenv

        try:
            from antenv.axon_hooks import get_axon_ntff_profile_hook  # noqa: F401

            return
        except ImportError:
            pass
        from trn_agent_boot.trn_boot import _ntff_profile_via_ctypes

        mod = types.ModuleType("antenv.axon_hooks")
        _st = {"hook": None}
        mod.set_axon_ntff_profile_hook = lambda h: _st.update(hook=h)
        mod.get_axon_ntff_profile_hook = lambda: _st["hook"]
        sys.modules["antenv.axon_hooks"] = mod
        antenv.axon_hooks = mod
        mod.set_axon_ntff_profile_hook(
            _ntff_profile_via_ctypes("/opt/axon/libaxon_pjrt.so")
        )
        # avoid S3 uploads from the trace post-processing in this container
        import concourse.bass_utils as _bu

        _bu.upload_artifacts = lambda tmpdir: tmpdir
    except Exception as e:  # profiling is best-effort
        print(f"ntff hook registration failed: {e}", file=sys.stderr)


def _shard(y: np.ndarray):
    counts = np.bincount(y, minlength=K).astype(np.int64)
    order = np.argsort(-counts, kind="stable")
    core_classes = [[] for _ in range(NCORES)]
    for rank, cls in enumerate(order):
        core_classes[rank % NCORES].append(int(cls))
    slot_sizes = [0] * CLS
    for j in range(NPAIR):
        m = 0
        for s in (2 * j, 2 * j + 1):
            for c in range(NCORES):
                m = max(m, int(counts[core_classes[c][s]]))
        S = m + 1  # guaranteed >=1 phantom column
        S = (S + 31) // 32 * 32  # 32-aligned so Gram row-splits are legal
        S = min(max(S, 128), 224)
        slot_sizes[2 * j] = S
        slot_sizes[2 * j + 1] = S
    mmax = int(counts.max())
    assert mmax + 1 <= 224, "class too large for paired psum layout"
    return counts, core_classes, tuple(slot_sizes)


def kernel(x: np.ndarray, y: np.ndarray) -> np.ndarray:
    x = np.ascontiguousarray(np.asarray(x, dtype=np.float32))
    y = np.asarray(y).astype(np.int64).ravel()
    N = x.shape[0]
    assert x.shape == (N, C)

    counts, core_classes, slot_sizes = _shard(y)
    R = int(sum(slot_sizes))
    slot_off = np.concatenate([[0], np.cumsum(slot_sizes)]).astype(np.int64)

    key = (R, slot_sizes)
    if key not in _nc_cache:
        _nc_cache[key] = _build_nc(slot_sizes, R)
    nc = _nc_cache[key]

    # ---- build per-core inputs ----
    xTfull = np.ascontiguousarray(x.T)  # [C, N]
    in_maps = []
    for j in range(NCORES):
        xt = np.zeros((128, NCH, R), dtype=np.float32)
        cnt = np.zeros((128, 5, NCH, CLS), dtype=np.float32)
        for s in range(CLS):
            cls = core_classes[j][s]
            idx = np.flatnonzero(y == cls)
            n = len(idx)
            S = slot_sizes[s]
            o = slot_off[s]
            if n:
                # [C, n] -> [4, 128, n] -> [128, 4, n]
                blk = xTfull[:, idx].reshape(NCH, 128, n).transpose(1, 0, 2)
                xt[:, :, o : o + n] = blk
            ce = (S + 1) // 2
            co = S // 2
            cnt[:, 0, :, s] = float(n)
            cnt[:, 1, :, s] = 1.0 / max(n, 1)
            cnt[:, 2, :, s] = 1.0 / max(n - 1, 1)
            cnt[:, 3, :, s] = float(ce)
            cnt[:, 4, :, s] = float(co)
        in_maps.append({"xt": xt, "cnt": cnt})

    trace = bool(int(os.environ.get("KERNEL_TRACE", "0")))
    if trace:
        _ensure_axon_ntff_hook()
    res = run_bass_kernel_spmd(
        nc,
        in_maps,
        core_ids=list(range(NCORES)),
        trace=trace,
        **({"trace_cores": [0], "stitch_traces": False} if trace else {}),
    )
    global _last_results
    _last_results = res

    # ---- host combine (the unshard/gather step) ----
    off_denom = np.float64(C * (C - 1))
    loss_num = np.float64(0.0)
    n_count = np.float64(0.0)
    for j in range(NCORES):
        o = np.asarray(res.results[j]["outv"], dtype=np.float64).reshape(OUTW)
        for s in range(CLS):
            cls = core_classes[j][s]
            n = int(counts[cls])
            if n <= 1:
                continue
            S = slot_sizes[s]
            n_pad = S - n
            gsq = sum(o[ACC0 + 3 * s + i] for i in range(3))
            dsum = sum(o[DSQ0 + 16 * ch + s] for ch in range(NCH))
            rho = sum(o[MURQ0 + 16 * ch + s] for ch in range(NCH))
            P1 = sum(o[P10 + 3 * s + i] for i in range(3))
            sqq = P1 - n_pad * rho * rho
            F = gsq - 2.0 * n_pad * sqq - (n_pad * n_pad) * rho * rho
            off_sum = F - dsum
            loss_num += off_sum / off_denom
            n_count += n
    if n_count > 0:
        out = loss_num / max(n_count, 1.0)
    else:
        out = 0.0
    return np.float32(out)



# revision 7
# speedup vs baseline: 1.0851x; 1.0851x over previous
"""Trainium2 Bass kernel for nn_DecorrelateLossClass (segment_reduce / ridge).

v2 strategy (class-sharded, collective-free, bf16 data path):
  * K=128 classes -> 16 per core (snake by descending count). Within a core
    the 16 classes are sorted ASCENDING by count into slot ranks r=0..15;
    slot r maps to group g=r%4, pos p=r//4, so the 4 pipeline groups have
    balanced widths and each PSUM bank sees ascending widths over time
    (stale-region safety for exact Frobenius reads).
  * Host packs x feature-major in bf16: xt[g][128, 4ch, GR], class slots
    padded with the CLASS MEAN so normalized pads are ~0 -> no phantom
    correction machinery at all.
  * Device: batched bn_stats (2 chunks x slot per op) -> per-group stats
    math on the Pool engine -> sqrt (ACT) + reciprocal (DVE) -> per-class
    normalization z=(x-mu)*r in bf16 (ACT via scale/bias, DVE via
    tensor_scalar) -> per-class exact Gram Z_k^T Z_k in bf16 on the PE
    (1 cycle/row vs fp32's 3.5) -> single-op Frobenius reduction per class
    (DVE tensor_tensor_reduce or ACT Square+accum over the [128,2,S] PSUM
    tile; rc1 tail rows are guaranteed zero by a PE zero-matmul + the
    ascending-width bank schedule).
  * diag(corr)^2 is analytic from the stats. Final partition reduce via
    ones-matmul; host sums 8x16 per-class scalars.
"""

import os
import sys

import numpy as np

for _p in ("/opt/trn_rl_repo",):
    if os.path.isdir(_p) and _p not in sys.path:
        sys.path.insert(0, _p)

import concourse.bass as bass
from concourse import bacc
import concourse.mybir as mybir
import concourse.tile as tile
from concourse.bass_utils import run_bass_kernel_spmd

import ml_dtypes

BF16 = ml_dtypes.bfloat16

K = 128
C = 512
NCH = 4  # feature chunks of 128
NCORES = 8
CLS = 16  # classes per core
NG = 4  # pipeline groups
GP = CLS // NG  # slots per group
EPS = 1e-8

# fin layout: gsq per stats-index t at cols [2t, 2t+1], dsq [32:96] (ch*16+t)
GSQ0, DSQ0, NF = 0, 32, 96

# engine assignment knobs (tuned from traces)
NORM_ENG = {0: "act", 1: "act", 2: "act", 3: "dve"}
SQ_ENG = {0: "split", 1: "split", 2: "act", 3: "act"}
NEWTON = False

_nc_cache: dict = {}
_last_results = None


def _build_nc(slot_sizes: tuple, GR: int):
    """slot_sizes indexed by rank r (ascending sizes); r = p*NG + g."""
    f32 = mybir.dt.float32
    bf16 = mybir.dt.bfloat16
    AF = mybir.ActivationFunctionType
    OP = mybir.AluOpType

    def S_of(g, p):
        return slot_sizes[p * NG + g]

    def t_of(g, p):
        return g * GP + p

    # column offsets within each group's buffer
    qoff = [[0] * GP for _ in range(NG)]
    for g in range(NG):
        acc = 0
        for p in range(GP):
            qoff[g][p] = acc
            acc += S_of(g, p)
        assert acc <= GR

    nc = bacc.Bacc("TRN2", target_bir_lowering=False)
    xt_d = nc.dram_tensor("xt", [NG, 128, NCH, GR], bf16, kind="ExternalInput")
    cnt_d = nc.dram_tensor("cnt", [128, 5, NCH, CLS], f32, kind="ExternalInput")
    out_d = nc.dram_tensor("outv", [1, NF], f32, kind="ExternalOutput")

    V = nc.vector
    A = nc.scalar
    P = nc.gpsimd
    T = nc.tensor

    with tile.TileContext(nc) as tc:
        with (
            tc.tile_pool(name="persist", bufs=1) as persist,
            tc.tile_pool(name="stats", bufs=1) as stats,
            tc.tile_pool(name="sqscr", bufs=2) as sqscr_pool,
            tc.tile_pool(name="gram", bufs=4, space="PSUM") as gram,
            tc.tile_pool(name="fpsum", bufs=1, space="PSUM") as fpsum,
        ):
            x_g = [
                persist.tile([128, NCH, GR], bf16, tag=f"x{g}", name=f"x{g}")
                for g in range(NG)
            ]
            z_g = [
                persist.tile([128, NCH, GR], bf16, tag=f"z{g}", name=f"z{g}")
                for g in range(NG)
            ]
            cnt_sb = persist.tile([128, 5, NCH, CLS], f32, tag="cnt")
            bnbuf = persist.tile([128, NCH, CLS, 6], f32, tag="bnbuf")
            fin = persist.tile([128, NF], f32, tag="fin")
            ones = persist.tile([128, 1], f32, tag="ones")
            zeros_bf = persist.tile([128, 512], bf16, tag="zbf")

            # ---- init (Pool engine; keep DVE clean) ----
            P.memset(fin, 0.0)
            P.memset(ones, 1.0)
            P.memset(zeros_bf, 0.0)

            # ---- DMA in ----
            nc.sync.dma_start(out=cnt_sb, in_=cnt_d[:, :, :, :])
            for g in range(NG):
                nc.sync.dma_start(out=x_g[g], in_=xt_d[g, :, :, :])

            # ---- stats tiles ----
            def st(tag):
                return stats.tile([128, NCH, CLS], f32, tag=tag, name=tag)

            t1 = st("t1")
            t2 = st("t2")
            s1 = st("s1")
            s2 = st("s2")
            mu = st("mu")
            m2 = st("m2")
            tt = st("tt")
            var = st("var")
            tv = st("tv")
            sq = st("sq")
            r = st("r")
            mur = st("mur")
            nmur = st("nmur")
            r2 = st("r2")
            d = st("d")
            if NEWTON:
                r0 = st("r0")
                w = st("w")

            me = bnbuf[:, :, :, 1]
            ve = bnbuf[:, :, :, 2]
            mo = bnbuf[:, :, :, 4]
            vo = bnbuf[:, :, :, 5]
            Svec = cnt_sb[:, 0, :, :]
            invS = cnt_sb[:, 1, :, :]
            in1v = cnt_sb[:, 2, :, :]
            cev = cnt_sb[:, 3, :, :]
            cov = cnt_sb[:, 4, :, :]

            dsq_view = fin[:, DSQ0 : DSQ0 + 64].rearrange("p (c k) -> p c k", c=NCH)

            # ---- emission helpers ----
            def emit_bn(g):
                for p in range(GP):
                    t = t_of(g, p)
                    S = S_of(g, p)
                    q = qoff[g][p]
                    for ch in range(NCH):
                        V.bn_stats(
                            out=bnbuf[:, ch, t, :],
                            in_=x_g[g][:, ch, q : q + S],
                        )

            def emit_stats_pool(g):
                gs = slice(t_of(g, 0), t_of(g, 0) + GP)

                def sl(ap):
                    return ap[:, :, gs]

                TT = P.tensor_tensor
                TT(out=sl(t1), in0=sl(me), in1=sl(cev), op=OP.mult)
                TT(out=sl(t2), in0=sl(mo), in1=sl(cov), op=OP.mult)
                TT(out=sl(s1), in0=sl(t1), in1=sl(t2), op=OP.add)
                TT(out=sl(s2), in0=sl(ve), in1=sl(vo), op=OP.add)
                TT(out=sl(t1), in0=sl(t1), in1=sl(me), op=OP.mult)  # e2
                TT(out=sl(t2), in0=sl(t2), in1=sl(mo), op=OP.mult)  # o2
                TT(out=sl(s2), in0=sl(s2), in1=sl(t1), op=OP.add)
                TT(out=sl(s2), in0=sl(s2), in1=sl(t2), op=OP.add)
                TT(out=sl(mu), in0=sl(s1), in1=sl(invS), op=OP.mult)
                TT(out=sl(m2), in0=sl(mu), in1=sl(mu), op=OP.mult)
                TT(out=sl(m2), in0=sl(m2), in1=sl(Svec), op=OP.mult)
                TT(out=sl(tt), in0=sl(s2), in1=sl(m2), op=OP.subtract)
                TT(out=sl(var), in0=sl(tt), in1=sl(in1v), op=OP.mult)
                P.tensor_scalar_max(out=sl(var), in0=sl(var), scalar1=0.0)
                P.tensor_scalar_add(out=sl(tv), in0=sl(var), scalar1=float(EPS))

            def emit_sqrt(g):
                gs = slice(t_of(g, 0), t_of(g, 0) + GP)
                A.sqrt(out=sq[:, :, gs], in_=tv[:, :, gs])

            def emit_recip(g):
                gs = slice(t_of(g, 0), t_of(g, 0) + GP)
                if NEWTON:
                    V.reciprocal(out=r0[:, :, gs], in_=sq[:, :, gs])
                else:
                    V.reciprocal(out=r[:, :, gs], in_=sq[:, :, gs])

            def emit_newton_pool(g):
                if not NEWTON:
                    return
                gs = slice(t_of(g, 0), t_of(g, 0) + GP)

                def sl(ap):
                    return ap[:, :, gs]

                P.tensor_tensor(out=sl(w), in0=sl(r0), in1=sl(r0), op=OP.mult)
                P.tensor_tensor(out=sl(w), in0=sl(tv), in1=sl(w), op=OP.mult)
                P.tensor_scalar(
                    out=sl(w), in0=sl(w), scalar1=-0.5, scalar2=1.5,
                    op0=OP.mult, op1=OP.add,
                )
                P.tensor_tensor(out=sl(r), in0=sl(r0), in1=sl(w), op=OP.mult)

            def emit_mur_pool(g):
                gs = slice(t_of(g, 0), t_of(g, 0) + GP)
                P.tensor_tensor(
                    out=mur[:, :, gs], in0=mu[:, :, gs], in1=r[:, :, gs],
                    op=OP.mult,
                )
                P.tensor_scalar_mul(
                    out=nmur[:, :, gs], in0=mur[:, :, gs], scalar1=-1.0
                )

            def emit_norm(g):
                eng = NORM_ENG[g]
                for p in range(GP):
                    t = t_of(g, p)
                    S = S_of(g, p)
                    q = qoff[g][p]
                    for ch in range(NCH):
                        zsl = z_g[g][:, ch, q : q + S]
                        xsl = x_g[g][:, ch, q : q + S]
                        if eng == "act":
                            A.activation(
                                out=zsl,
                                in_=xsl,
                                func=AF.Identity,
                                scale=r[:, ch, t : t + 1],
                                bias=nmur[:, ch, t : t + 1],
                            )
                        elif eng == "dve":
                            V.tensor_scalar(
                                out=zsl,
                                in0=xsl,
                                scalar1=mu[:, ch, t : t + 1],
                                scalar2=r[:, ch, t : t + 1],
                                op0=OP.subtract,
                                op1=OP.mult,
                            )
                        else:
                            P.tensor_scalar(
                                out=zsl,
                                in0=xsl,
                                scalar1=mu[:, ch, t : t + 1],
                                scalar2=r[:, ch, t : t + 1],
                                op0=OP.subtract,
                                op1=OP.mult,
                            )

            ps_tiles = {}

            def emit_gram(g, p):
                t = t_of(g, p)
                S = S_of(g, p)
                q = qoff[g][p]
                ps = gram.tile([128, 2, 256], f32, tag="ps", name=f"ps{g}_{p}")
                ps_tiles[(g, p)] = ps
                if g == 0:
                    # zero the whole bank once (start of its ascending chain)
                    T.matmul(
                        ps[:, :, :],
                        lhsT=zeros_bf[:, 0:128],
                        rhs=zeros_bf[:, 0:512],
                        start=True,
                        stop=True,
                    )
                # rc0: rows 0:min(128,S)
                m0 = min(128, S)
                for ch in range(NCH):
                    T.matmul(
                        ps[0:m0, 0, 0:S],
                        lhsT=z_g[g][:, ch, q : q + m0],
                        rhs=z_g[g][:, ch, q : q + S],
                        start=(ch == 0),
                        stop=(ch == NCH - 1),
                    )
                if S > 128:
                    m1 = S - 128
                    for ch in range(NCH):
                        T.matmul(
                            ps[0:m1, 1, 0:S],
                            lhsT=z_g[g][:, ch, q + 128 : q + S],
                            rhs=z_g[g][:, ch, q : q + S],
                            start=(ch == 0),
                            stop=(ch == NCH - 1),
                        )

            def emit_square(g, p):
                t = t_of(g, p)
                S = S_of(g, p)
                ps = ps_tiles[(g, p)]
                scr = sqscr_pool.tile(
                    [128, 2, 256], bf16, tag="sqs", name=f"sqs{g}_{p}"
                )
                if SQ_ENG[g] == "split":
                    # ACT squares into SBUF bf16; DVE does the reduction
                    A.activation(
                        out=scr[:, :, 0:S],
                        in_=ps[:, :, 0:S],
                        func=AF.Square,
                    )
                    V.tensor_reduce(
                        out=fin[:, GSQ0 + 2 * t : GSQ0 + 2 * t + 2],
                        in_=scr[:, :, 0:S],
                        axis=mybir.AxisListType.X,
                        op=OP.add,
                    )
                else:
                    A.activation(
                        out=scr[:, :, 0:S],
                        in_=ps[:, :, 0:S],
                        func=AF.Square,
                        accum_out=fin[:, GSQ0 + 2 * t : GSQ0 + 2 * t + 1],
                    )

            # =========== emission schedule ===========
            # DVE: bn(g0) bn(g1) rec(g0) bn(g2) rec(g1) bn(g3) [sq g0]
            #      rec(g2) [sq g1] rec(g3) [norm g3] [sq rest]
            # Pool: stats(g0..g3) + mur chains
            # ACT: sqrt(g) + norms g0..g2 + squares g2/g3
            emit_bn(0)
            emit_bn(1)
            emit_stats_pool(0)
            emit_sqrt(0)
            emit_recip(0)
            emit_newton_pool(0)
            if NORM_ENG[0] == "act":
                emit_mur_pool(0)
            emit_bn(2)
            emit_stats_pool(1)
            emit_sqrt(1)
            emit_recip(1)
            emit_newton_pool(1)
            if NORM_ENG[1] == "act":
                emit_mur_pool(1)
            emit_norm(0)
            for p in range(GP):
                emit_gram(0, p)
            emit_bn(3)
            emit_stats_pool(2)
            emit_sqrt(2)
            emit_recip(2)
            emit_newton_pool(2)
            if NORM_ENG[2] == "act":
                emit_mur_pool(2)
            emit_norm(1)
            for p in range(GP):
                emit_square(0, p)
                emit_gram(1, p)
            emit_stats_pool(3)
            emit_sqrt(3)
            emit_recip(3)
            emit_newton_pool(3)
            if NORM_ENG[3] == "act":
                emit_mur_pool(3)
            emit_norm(2)
            for p in range(GP):
                emit_square(1, p)
                emit_gram(2, p)
            emit_norm(3)
            for p in range(GP):
                emit_square(2, p)
                emit_gram(3, p)
            for p in range(GP):
                emit_square(3, p)

            # lazy diag chain (Pool): d = tt*r^2 ; dsq = d*d -> fin
            P.tensor_tensor(out=r2, in0=r, in1=r, op=OP.mult)
            P.tensor_tensor(out=d, in0=tt, in1=r2, op=OP.mult)
            P.tensor_tensor(out=dsq_view, in0=d, in1=d, op=OP.mult)

            # ---- final partition reduction via ones-matmul ----
            fps = fpsum.tile([1, NF], f32, tag="fps")
            T.matmul(fps, lhsT=ones, rhs=fin, start=True, stop=True)
            outsb = persist.tile([1, NF], f32, tag="outsb")
            V.tensor_copy(out=outsb, in_=fps)
            nc.sync.dma_start(out=out_d[:, :], in_=outsb)

    nc.compile()
    return nc


def _ensure_axon_ntff_hook():
    """Register the axon NTFF profiling hook if the image's antenv lacks it."""
    try:
        import types

        import antenv

        try:
            from antenv.axon_hooks import get_axon_ntff_profile_hook  # noqa: F401

            return
        except ImportError:
            pass
        from trn_agent_boot.trn_boot import _ntff_profile_via_ctypes

        mod = types.ModuleType("antenv.axon_hooks")
        _st = {"hook": None}
        mod.set_axon_ntff_profile_hook = lambda h: _st.update(hook=h)
        mod.get_axon_ntff_profile_hook = lambda: _st["hook"]
        sys.modules["antenv.axon_hooks"] = mod
        antenv.axon_hooks = mod
        mod.set_axon_ntff_profile_hook(
            _ntff_profile_via_ctypes("/opt/axon/libaxon_pjrt.so")
        )
        import concourse.bass_utils as _bu

        _bu.upload_artifacts = lambda tmpdir: tmpdir
    except Exception as e:  # profiling is best-effort
        print(f"ntff hook registration failed: {e}", file=sys.stderr)


def _shard(y: np.ndarray):
    counts = np.bincount(y, minlength=K).astype(np.int64)
    order = np.argsort(-counts, kind="stable")
    core_classes = [[] for _ in range(NCORES)]
    for i, cls in enumerate(order):
        row, col = i // NCORES, i % NCORES
        core = col if row % 2 == 0 else NCORES - 1 - col
        core_classes[core].append(int(cls))
    # sort each core's classes ascending by count -> rank r
    for c in range(NCORES):
        core_classes[c].sort(key=lambda k: counts[k])
    slot_sizes = [0] * CLS
    for rank in range(CLS):
        m = max(int(counts[core_classes[c][rank]]) for c in range(NCORES))
        S = max(m, 2)
        S = (S + 1) // 2 * 2  # even, for 4B-aligned bf16 slices
        assert S <= 256, "class too large for psum bank layout"
        slot_sizes[rank] = S
    assert all(
        slot_sizes[rank] <= slot_sizes[rank + 1] for rank in range(CLS - 1)
    )
    return counts, core_classes, tuple(slot_sizes)


def kernel(x: np.ndarray, y: np.ndarray) -> np.ndarray:
    x = np.ascontiguousarray(np.asarray(x, dtype=np.float32))
    y = np.asarray(y).astype(np.int64).ravel()
    N = x.shape[0]
    assert x.shape == (N, C)

    counts, core_classes, slot_sizes = _shard(y)

    def S_of(g, p):
        return slot_sizes[p * NG + g]

    GR = max(sum(S_of(g, p) for p in range(GP)) for g in range(NG))
    GR = (GR + 7) // 8 * 8
    qoff = [[0] * GP for _ in range(NG)]
    for g in range(NG):
        acc = 0
        for p in range(GP):
            qoff[g][p] = acc
            acc += S_of(g, p)

    key = (GR, slot_sizes)
    if key not in _nc_cache:
        _nc_cache[key] = _build_nc(slot_sizes, GR)
    nc = _nc_cache[key]

    # ---- build per-core inputs ----
    xTfull = np.ascontiguousarray(x.T)  # [C, N]
    in_maps = []
    for j in range(NCORES):
        xt = np.zeros((NG, 128, NCH, GR), dtype=np.float32)
        cnt = np.zeros((128, 5, NCH, CLS), dtype=np.float32)
        for rank in range(CLS):
            cls = core_classes[j][rank]
            g, p = rank % NG, rank // NG
            t = g * GP + p
            S = slot_sizes[rank]
            q = qoff[g][p]
            idx = np.flatnonzero(y == cls)
            n = len(idx)
            if n:
                blk = xTfull[:, idx].reshape(NCH, 128, n).transpose(1, 0, 2)
                xt[g, :, :, q : q + n] = blk
                if n < S:
                    muf = xTfull[:, idx].mean(axis=1)  # [C]
                    mu128 = muf.reshape(NCH, 128).T  # [128, NCH]
                    xt[g, :, :, q + n : q + S] = mu128[:, :, None]
            cnt[:, 0, :, t] = float(S)
            cnt[:, 1, :, t] = 1.0 / S
            cnt[:, 2, :, t] = 1.0 / max(n - 1, 1)
            cnt[:, 3, :, t] = float((S + 1) // 2)
            cnt[:, 4, :, t] = float(S // 2)
        in_maps.append({"xt": xt.astype(BF16), "cnt": cnt})

    trace = bool(int(os.environ.get("KERNEL_TRACE", "0")))
    if trace:
        _ensure_axon_ntff_hook()
    res = run_bass_kernel_spmd(
        nc,
        in_maps,
        core_ids=list(range(NCORES)),
        trace=trace,
        **({"trace_cores": [0], "stitch_traces": False} if trace else {}),
    )
    global _last_results
    _last_results = res

    # ---- host combine ----
    off_denom = np.float64(C * (C - 1))
    loss_num = np.float64(0.0)
    n_count = np.float64(0.0)
    for j in range(NCORES):
        o = np.asarray(res.results[j]["outv"], dtype=np.float64).reshape(NF)
        for rank in range(CLS):
            cls = core_classes[j][rank]
            n = int(counts[cls])
            if n <= 1:
                continue
            g, p = rank % NG, rank // NG
            t = g * GP + p
            gsq = o[GSQ0 + 2 * t] + o[GSQ0 + 2 * t + 1]
            dsum = sum(o[DSQ0 + ch * CLS + t] for ch in range(NCH))
            off_sum = gsq - dsum
            loss_num += off_sum / off_denom
            n_count += n
    out = loss_num / n_count if n_count > 0 else 0.0
    return np.float32(out)


# revision 16
# speedup vs baseline: 1.5453x; 1.4241x over previous
"""Trainium2 Bass kernel for nn_DecorrelateLossClass (segment_reduce / ridge).

Class-sharded, collective-free, bf16 data path:
  * 128 classes -> 16 per core (snake by descending count); within a core
    classes sort ASCENDING into slot ranks r (r = p*4+g), giving 4 balanced
    pipeline groups and ascending per-PSUM-bank widths (stale-region
    safety for the Frobenius reads).
  * Host packs x feature-major bf16 per group; class slots are padded with
    the CLASS MEAN so normalized pads are ~0 (no phantom corrections).
  * Device pipeline per group: bn_stats (DVE, even/odd fields give padded
    sums) -> Welford-merge stats math (Pool) -> sqrt (ACT) + reciprocal
    (DVE) -> z=(x-mu)*r in bf16 (ACT for early groups, DVE 4x-mode for
    late ones) -> per-class exact Gram Z_k^T Z_k (PE, bf16) -> Frobenius
    via ACT Square+accum (PSUM pads are guaranteed zero by a PE
    zero-matmul + ascending bank schedule).
  * diag(corr)^2 analytic from stats (Pool); ones-matmul partition
    reduce; host sums 8x16 per-class scalars.
  * Every stats tile is split per group to avoid whole-tile WAR/WAW
    false serialization; a dummy sqrt preloads the ACT table; PE warmup
    matmuls ramp the clock during the DMA prologue.
"""

import os
import sys

import numpy as np

for _p in ("/opt/trn_rl_repo",):
    if os.path.isdir(_p) and _p not in sys.path:
        sys.path.insert(0, _p)

import concourse.bass as bass
from concourse import bacc
import concourse.mybir as mybir
import concourse.tile as tile
from concourse.bass_utils import run_bass_kernel_spmd

import ml_dtypes

BF16 = ml_dtypes.bfloat16

K = 128
C = 512
NCH = 4  # feature chunks of 128
NCORES = 8
CLS = 16  # classes per core
NG = 4  # pipeline groups
GP = CLS // NG  # slots per group
EPS = 1e-8

# fin layout: gsq per stats-index t at cols [2t, 2t+1], dsq [32:96] (ch*16+t)
GSQ0, DSQ0, NF = 0, 16, 96

# engine assignment knobs (tuned from traces)
# norm engine per group: ACT handles early groups (DVE busy with bn_stats),
# DVE takes the late ones once bn is drained.
# slots (p indices) normalized on DVE per group; the rest go to ACT
NORM_DVE_SLOTS = {0: (), 1: (), 2: (0, 1, 2, 3), 3: (0, 1, 2, 3)}
N_WARMUP_MM = 24  # junk matmuls to ramp the PE clock before real grams
NEWTON = False

_nc_cache: dict = {}
_last_results = None


def _build_nc(slot_sizes: tuple, GR: int):
    """slot_sizes indexed by rank r (ascending sizes); r = p*NG + g."""
    f32 = mybir.dt.float32
    bf16 = mybir.dt.bfloat16
    AF = mybir.ActivationFunctionType
    OP = mybir.AluOpType

    def S_of(g, p):
        return slot_sizes[p * NG + g]

    def t_of(g, p):
        return g * GP + p

    # column offsets within each group's buffer
    qoff = [[0] * GP for _ in range(NG)]
    for g in range(NG):
        acc = 0
        for p in range(GP):
            qoff[g][p] = acc
            acc += S_of(g, p)
        assert acc <= GR

    nc = bacc.Bacc("TRN2", target_bir_lowering=False)
    xt_d = nc.dram_tensor("xt", [NG, 128, NCH, GR], bf16, kind="ExternalInput")
    cnt_d = nc.dram_tensor("cnt", [128, 5, NCH, CLS], f32, kind="ExternalInput")
    out_d = nc.dram_tensor("outv", [1, NF], f32, kind="ExternalOutput")

    V = nc.vector
    A = nc.scalar
    P = nc.gpsimd
    T = nc.tensor

    with tile.TileContext(nc) as tc:
        with (
            tc.tile_pool(name="persist", bufs=1) as persist,
            tc.tile_pool(name="stats", bufs=1) as stats,
            tc.tile_pool(name="sqscr", bufs=2) as sqscr_pool,
            tc.tile_pool(name="gram", bufs=4, space="PSUM") as gram,
            tc.tile_pool(name="fpsum", bufs=1, space="PSUM") as fpsum,
        ):
            x_g = [
                persist.tile([128, NCH, GR], bf16, tag=f"x{g}", name=f"x{g}")
                for g in range(NG)
            ]
            z_g = [
                persist.tile([128, NCH, GR], bf16, tag=f"z{g}", name=f"z{g}")
                for g in range(NG)
            ]
            cnt_sb = persist.tile([128, 5, NCH, CLS], f32, tag="cnt")
            bnbuf = persist.tile([128, NCH, CLS, 6], f32, tag="bnbuf")
            fin = persist.tile([128, NF], f32, tag="fin")
            ones = persist.tile([128, 1], f32, tag="ones")
            zeros_bf = persist.tile([128, 512], bf16, tag="zbf")

            # ---- init (Pool engine; keep DVE clean) ----
            P.memset(fin, 0.0)
            P.memset(ones, 1.0)
            P.memset(zeros_bf, 0.0)

            # ---- DMA in ----
            nc.sync.dma_start(out=cnt_sb, in_=cnt_d[:, :, :, :])
            for g in range(NG):
                nc.sync.dma_start(out=x_g[g], in_=xt_d[g, :, :, :])

            # ---- stats tiles ----
            def st(tag):
                return stats.tile([128, NCH, CLS], f32, tag=tag, name=tag)

            t1 = st("t1")
            t2 = st("t2")
            s1 = st("s1")
            s2 = st("s2")
            mu = st("mu")
            m2 = st("m2")
            tt = st("tt")
            var = st("var")
            tv = st("tv")
            sq = st("sq")
            r = st("r")
            mur = st("mur")
            nmur = st("nmur")
            r2 = st("r2")
            d = st("d")
            if NEWTON:
                r0 = st("r0")
                w = st("w")

            me = bnbuf[:, :, :, 1]
            ve = bnbuf[:, :, :, 2]
            mo = bnbuf[:, :, :, 4]
            vo = bnbuf[:, :, :, 5]
            Svec = cnt_sb[:, 0, :, :]
            invS = cnt_sb[:, 1, :, :]
            in1v = cnt_sb[:, 2, :, :]
            cev = cnt_sb[:, 3, :, :]
            cov = cnt_sb[:, 4, :, :]

            dsq_view = fin[:, DSQ0 : DSQ0 + 64].rearrange("p (c k) -> p c k", c=NCH)

            # ---- emission helpers ----
            def emit_bn(g):
                for p in range(GP):
                    t = t_of(g, p)
                    S = S_of(g, p)
                    q = qoff[g][p]
                    for ch in range(NCH):
                        V.bn_stats(
                            out=bnbuf[:, ch, t, :],
                            in_=x_g[g][:, ch, q : q + S],
                        )

            def emit_stats_pool(g):
                gs = slice(t_of(g, 0), t_of(g, 0) + GP)

                def sl(ap):
                    return ap[:, :, gs]

                TT = P.tensor_tensor
                TT(out=sl(t1), in0=sl(me), in1=sl(cev), op=OP.mult)
                TT(out=sl(t2), in0=sl(mo), in1=sl(cov), op=OP.mult)
                TT(out=sl(s1), in0=sl(t1), in1=sl(t2), op=OP.add)
                TT(out=sl(s2), in0=sl(ve), in1=sl(vo), op=OP.add)
                TT(out=sl(t1), in0=sl(t1), in1=sl(me), op=OP.mult)  # e2
                TT(out=sl(t2), in0=sl(t2), in1=sl(mo), op=OP.mult)  # o2
                TT(out=sl(s2), in0=sl(s2), in1=sl(t1), op=OP.add)
                TT(out=sl(s2), in0=sl(s2), in1=sl(t2), op=OP.add)
                TT(out=sl(mu), in0=sl(s1), in1=sl(invS), op=OP.mult)
                TT(out=sl(m2), in0=sl(mu), in1=sl(mu), op=OP.mult)
                TT(out=sl(m2), in0=sl(m2), in1=sl(Svec), op=OP.mult)
                TT(out=sl(tt), in0=sl(s2), in1=sl(m2), op=OP.subtract)
                TT(out=sl(var), in0=sl(tt), in1=sl(in1v), op=OP.mult)
                P.tensor_scalar_max(out=sl(var), in0=sl(var), scalar1=0.0)
                P.tensor_scalar_add(out=sl(tv), in0=sl(var), scalar1=float(EPS))

            def emit_sqrt(g):
                gs = slice(t_of(g, 0), t_of(g, 0) + GP)
                A.sqrt(out=sq[:, :, gs], in_=tv[:, :, gs])

            def emit_recip(g):
                gs = slice(t_of(g, 0), t_of(g, 0) + GP)
                if NEWTON:
                    V.reciprocal(out=r0[:, :, gs], in_=sq[:, :, gs])
                else:
                    V.reciprocal(out=r[:, :, gs], in_=sq[:, :, gs])

            def emit_newton_pool(g):
                if not NEWTON:
                    return
                gs = slice(t_of(g, 0), t_of(g, 0) + GP)

                def sl(ap):
                    return ap[:, :, gs]

                P.tensor_tensor(out=sl(w), in0=sl(r0), in1=sl(r0), op=OP.mult)
                P.tensor_tensor(out=sl(w), in0=sl(tv), in1=sl(w), op=OP.mult)
                P.tensor_scalar(
                    out=sl(w), in0=sl(w), scalar1=-0.5, scalar2=1.5,
                    op0=OP.mult, op1=OP.add,
                )
                P.tensor_tensor(out=sl(r), in0=sl(r0), in1=sl(w), op=OP.mult)

            def emit_mur_pool(g):
                gs = slice(t_of(g, 0), t_of(g, 0) + GP)
                P.tensor_tensor(
                    out=mur[:, :, gs], in0=mu[:, :, gs], in1=r[:, :, gs],
                    op=OP.mult,
                )
                P.tensor_scalar_mul(
                    out=nmur[:, :, gs], in0=mur[:, :, gs], scalar1=-1.0
                )

            def emit_norm(g):
                eng = NORM_ENG[g]
                for p in range(GP):
                    t = t_of(g, p)
                    S = S_of(g, p)
                    q = qoff[g][p]
                    for ch in range(NCH):
                        zsl = z_g[g][:, ch, q : q + S]
                        xsl = x_g[g][:, ch, q : q + S]
                        if eng == "act":
                            A.activation(
                                out=zsl,
                                in_=xsl,
                                func=AF.Identity,
                                scale=r[:, ch, t : t + 1],
                                bias=nmur[:, ch, t : t + 1],
                            )
                        elif eng == "dve":
                            V.tensor_scalar(
                                out=zsl,
                                in0=xsl,
                                scalar1=mu[:, ch, t : t + 1],
                                scalar2=r[:, ch, t : t + 1],
                                op0=OP.subtract,
                                op1=OP.mult,
                            )
                        else:
                            P.tensor_scalar(
                                out=zsl,
                                in0=xsl,
                                scalar1=mu[:, ch, t : t + 1],
                                scalar2=r[:, ch, t : t + 1],
                                op0=OP.subtract,
                                op1=OP.mult,
                            )

            ps_tiles = {}

            def emit_gram(g, p):
                t = t_of(g, p)
                S = S_of(g, p)
                q = qoff[g][p]
                ps = gram.tile([128, 2, 256], f32, tag="ps", name=f"ps{g}_{p}")
                ps_tiles[(g, p)] = ps
                if g == 0:
                    # zero the whole bank once (start of its ascending chain)
                    T.matmul(
                        ps[:, :, :],
                        lhsT=zeros_bf[:, 0:128],
                        rhs=zeros_bf[:, 0:512],
                        start=True,
                        stop=True,
                    )
                # rc0: rows 0:min(128,S)
                m0 = min(128, S)
                for ch in range(NCH):
                    T.matmul(
                        ps[0:m0, 0, 0:S],
                        lhsT=z_g[g][:, ch, q : q + m0],
                        rhs=z_g[g][:, ch, q : q + S],
                        start=(ch == 0),
                        stop=(ch == NCH - 1),
                    )
                if S > 128:
                    m1 = S - 128
                    for ch in range(NCH):
                        T.matmul(
                            ps[0:m1, 1, 0:S],
                            lhsT=z_g[g][:, ch, q + 128 : q + S],
                            rhs=z_g[g][:, ch, q : q + S],
                            start=(ch == 0),
                            stop=(ch == NCH - 1),
                        )

            def emit_square(g, p):
                t = t_of(g, p)
                S = S_of(g, p)
                ps = ps_tiles[(g, p)]
                scr = sqscr_pool.tile(
                    [128, 2, 256], bf16, tag="sqs", name=f"sqs{g}_{p}"
                )
                if SQ_ENG[g] == "split":
                    # ACT squares into SBUF bf16; DVE does the reduction
                    A.activation(
                        out=scr[:, :, 0:S],
                        in_=ps[:, :, 0:S],
                        func=AF.Square,
                    )
                    V.tensor_reduce(
                        out=fin[:, GSQ0 + 2 * t : GSQ0 + 2 * t + 2],
                        in_=scr[:, :, 0:S],
                        axis=mybir.AxisListType.X,
                        op=OP.add,
                    )
                else:
                    A.activation(
                        out=scr[:, :, 0:S],
                        in_=ps[:, :, 0:S],
                        func=AF.Square,
                        accum_out=fin[:, GSQ0 + 2 * t : GSQ0 + 2 * t + 1],
                    )

            # =========== emission schedule ===========
            # DVE: bn0 bn1 rec0 nmur0 n0d bn2 rec1 nmur1 n1d bn3 rec2 n2d
            #      rec3 n3d
            # ACT: sqrt0 n0a sqrt1 n1a sq0 sqrt2 sq1 sqrt3 sq2 sq3
            # Pool: stats0..3 dsq
            def dve_ps(g):
                return NORM_DVE_SLOTS[g]

            def act_ps(g):
                return tuple(p for p in range(GP) if p not in NORM_DVE_SLOTS[g])

            emit_bn(0)
            emit_bn(1)
            emit_stats_pool(0)
            emit_sqrt(0)
            emit_recip(0)
            if act_ps(0):
                emit_nmur_dve(0)
            emit_stats_pool(1)
            emit_norm(0, "dve", ps=dve_ps(0))
            emit_norm(0, "act", ps=act_ps(0))
            for p in range(GP):
                emit_gram(0, p)
            emit_bn(2)
            emit_sqrt(1)
            emit_recip(1)
            if act_ps(1):
                emit_nmur_dve(1)
            emit_stats_pool(2)
            emit_norm(1, "dve", ps=dve_ps(1))
            emit_norm(1, "act", ps=act_ps(1))
            for p in range(GP):
                emit_gram(1, p)
            emit_bn(3)
            emit_sqrt(2)
            emit_recip(2)
            if act_ps(2):
                emit_nmur_dve(2)
            emit_stats_pool(3)
            emit_norm(2, "dve", ps=dve_ps(2))
            emit_norm(2, "act", ps=act_ps(2))
            for p in range(GP):
                emit_gram(2, p)
                emit_square(0, p)
            emit_sqrt(3)
            emit_recip(3)
            if act_ps(3):
                emit_nmur_dve(3)
            emit_norm(3, "dve", ps=dve_ps(3))
            emit_norm(3, "act", ps=act_ps(3))
            for p in range(GP):
                emit_square(1, p)
            for p in range(GP):
                emit_gram(3, p)
                emit_square(2, p)
            for p in range(GP):
                emit_square(3, p)

            # lazy diag chain (Pool): d = tt*r^2 ; dsq = d*d -> fin
            P.tensor_tensor(out=r2, in0=r, in1=r, op=OP.mult)
            P.tensor_tensor(out=d, in0=tt, in1=r2, op=OP.mult)
            P.tensor_tensor(out=dsq_view, in0=d, in1=d, op=OP.mult)

            # ---- final partition reduction via ones-matmul ----
            fps = fpsum.tile([1, NF], f32, tag="fps")
            T.matmul(fps, lhsT=ones, rhs=fin, start=True, stop=True)
            outsb = persist.tile([1, NF], f32, tag="outsb")
            V.tensor_copy(out=outsb, in_=fps)
            nc.sync.dma_start(out=out_d[:, :], in_=outsb)

    nc.compile()
    return nc


def _ensure_axon_ntff_hook():
    """Register the axon NTFF profiling hook if the image's antenv lacks it."""
    try:
        import types

        import antenv

        try:
            from antenv.axon_hooks import get_axon_ntff_profile_hook  # noqa: F401

            return
        except ImportError:
            pass
        from trn_agent_boot.trn_boot import _ntff_profile_via_ctypes

        mod = types.ModuleType("antenv.axon_hooks")
        _st = {"hook": None}
        mod.set_axon_ntff_profile_hook = lambda h: _st.update(hook=h)
        mod.get_axon_ntff_profile_hook = lambda: _st["hook"]
        sys.modules["antenv.axon_hooks"] = mod
        antenv.axon_hooks = mod
        mod.set_axon_ntff_profile_hook(
            _ntff_profile_via_ctypes("/opt/axon/libaxon_pjrt.so")
        )
        import concourse.bass_utils as _bu

        _bu.upload_artifacts = lambda tmpdir: tmpdir
    except Exception as e:  # profiling is best-effort
        print(f"ntff hook registration failed: {e}", file=sys.stderr)


def _shard(y: np.ndarray):
    counts = np.bincount(y, minlength=K).astype(np.int64)
    order = np.argsort(-counts, kind="stable")
    core_classes = [[] for _ in range(NCORES)]
    for i, cls in enumerate(order):
        row, col = i // NCORES, i % NCORES
        core = col if row % 2 == 0 else NCORES - 1 - col
        core_classes[core].append(int(cls))
    # sort each core's classes ascending by count -> rank r
    for c in range(NCORES):
        core_classes[c].sort(key=lambda k: counts[k])
    slot_sizes = [0] * CLS
    for rank in range(CLS):
        m = max(int(counts[core_classes[c][rank]]) for c in range(NCORES))
        S = max(m, 2)
        S = (S + 1) // 2 * 2  # even, for 4B-aligned bf16 slices
        assert S <= 256, "class too large for psum bank layout"
        slot_sizes[rank] = S
    assert all(
        slot_sizes[rank] <= slot_sizes[rank + 1] for rank in range(CLS - 1)
    )
    return counts, core_classes, tuple(slot_sizes)


def kernel(x: np.ndarray, y: np.ndarray) -> np.ndarray:
    x = np.ascontiguousarray(np.asarray(x, dtype=np.float32))
    y = np.asarray(y).astype(np.int64).ravel()
    N = x.shape[0]
    assert x.shape == (N, C)

    counts, core_classes, slot_sizes = _shard(y)

    def S_of(g, p):
        return slot_sizes[p * NG + g]

    GR = max(sum(S_of(g, p) for p in range(GP)) for g in range(NG))
    GR = (GR + 7) // 8 * 8
    qoff = [[0] * GP for _ in range(NG)]
    for g in range(NG):
        acc = 0
        for p in range(GP):
            qoff[g][p] = acc
            acc += S_of(g, p)

    key = (GR, slot_sizes)
    if key not in _nc_cache:
        _nc_cache[key] = _build_nc(slot_sizes, GR)
    nc = _nc_cache[key]

    # ---- build per-core inputs ----
    xTfull = np.ascontiguousarray(x.T)  # [C, N]
    in_maps = []
    for j in range(NCORES):
        xt = np.zeros((NG, 128, NCH, GR), dtype=np.float32)
        cnt = np.zeros((128, 5, NCH, CLS), dtype=np.float32)
        for rank in range(CLS):
            cls = core_classes[j][rank]
            g, p = rank % NG, rank // NG
            t = g * GP + p
            S = slot_sizes[rank]
            q = qoff[g][p]
            idx = np.flatnonzero(y == cls)
            n = len(idx)
            if n:
                blk = xTfull[:, idx].reshape(NCH, 128, n).transpose(1, 0, 2)
                xt[g, :, :, q : q + n] = blk
                if n < S:
                    muf = xTfull[:, idx].mean(axis=1)  # [C]
                    mu128 = muf.reshape(NCH, 128).T  # [128, NCH]
                    xt[g, :, :, q + n : q + S] = mu128[:, :, None]
            ce, co = (S + 1) // 2, S // 2
            cnt[:, 0, :, t] = ce / S
            cnt[:, 1, :, t] = co / S
            cnt[:, 2, :, t] = ce * co / S
            cnt[:, 3, :, t] = 1.0 / max(n - 1, 1)
        in_maps.append({"xt": xt.astype(BF16), "cnt": cnt})

    trace = bool(int(os.environ.get("KERNEL_TRACE", "0")))
    if trace:
        _ensure_axon_ntff_hook()
    res = run_bass_kernel_spmd(
        nc,
        in_maps,
        core_ids=list(range(NCORES)),
        trace=trace,
        **({"trace_cores": [0], "stitch_traces": False} if trace else {}),
    )
    global _last_results
    _last_results = res

    # ---- host combine ----
    off_denom = np.float64(C * (C - 1))
    loss_num = np.float64(0.0)
    n_count = np.float64(0.0)
    for j in range(NCORES):
        o = np.asarray(res.results[j]["outv"], dtype=np.float64).reshape(NF)
        for rank in range(CLS):
            cls = core_classes[j][rank]
            n = int(counts[cls])
            if n <= 1:
                continue
            g, p = rank % NG, rank // NG
            t = g * GP + p
            gsq = o[GSQ0 + t]
            dsum = sum(o[DSQ0 + ch * CLS + t] for ch in range(NCH))
            off_sum = gsq - dsum
            loss_num += off_sum / off_denom
            n_count += n
    out = loss_num / n_count if n_count > 0 else 0.0
    return np.float32(out)


# revision 18
# speedup vs baseline: 1.5556x; 1.0067x over previous
"""Trainium2 Bass kernel for nn_DecorrelateLossClass (segment_reduce / ridge).

Class-sharded, collective-free, bf16 data path:
  * 128 classes -> 16 per core (snake by descending count); within a core
    classes sort ASCENDING into slot ranks r (r = p*4+g), giving 4 balanced
    pipeline groups and ascending per-PSUM-bank widths (stale-region
    safety for the Frobenius reads).
  * Host packs x feature-major bf16 per group; class slots are padded with
    the CLASS MEAN so normalized pads are ~0 (no phantom corrections).
  * Device pipeline per group: bn_stats (DVE, even/odd fields give padded
    sums) -> Welford-merge stats math (Pool) -> sqrt (ACT) + reciprocal
    (DVE) -> z=(x-mu)*r in bf16 (ACT for early groups, DVE 4x-mode for
    late ones) -> per-class exact Gram Z_k^T Z_k (PE, bf16) -> Frobenius
    via ACT Square+accum (PSUM pads are guaranteed zero by a PE
    zero-matmul + ascending bank schedule).
  * diag(corr)^2 analytic from stats (Pool); ones-matmul partition
    reduce; host sums 8x16 per-class scalars.
  * Every stats tile is split per group to avoid whole-tile WAR/WAW
    false serialization; a dummy sqrt preloads the ACT table; PE warmup
    matmuls ramp the clock during the DMA prologue.
"""

import os
import sys

import numpy as np

for _p in ("/opt/trn_rl_repo",):
    if os.path.isdir(_p) and _p not in sys.path:
        sys.path.insert(0, _p)

import concourse.bass as bass
from concourse import bacc
import concourse.mybir as mybir
import concourse.tile as tile
from concourse.bass_utils import run_bass_kernel_spmd

import ml_dtypes

BF16 = ml_dtypes.bfloat16

K = 128
C = 512
NCH = 4  # feature chunks of 128
NCORES = 8
CLS = 16  # classes per core
NG = 4  # pipeline groups
GP = CLS // NG  # slots per group
EPS = 1e-8

# fin layout: gsq per stats-index t at cols [2t, 2t+1], dsq [32:96] (ch*16+t)
GSQ0, DSQ0, NF = 0, 16, 96

# engine assignment knobs (tuned from traces)
# norm engine per group: ACT handles early groups (DVE busy with bn_stats),
# DVE takes the late ones once bn is drained.
# slots (p indices) normalized on DVE per group; the rest go to ACT
NORM_DVE_SLOTS = {0: (), 1: (), 2: (0, 1, 2, 3), 3: (0, 1, 2, 3)}
N_WARMUP_MM = 24  # junk matmuls to ramp the PE clock before real grams
NEWTON = False

_nc_cache: dict = {}
_last_results = None


def _build_nc(slot_sizes: tuple, GR: int):
    """slot_sizes indexed by rank r (ascending sizes); r = p*NG + g."""
    f32 = mybir.dt.float32
    bf16 = mybir.dt.bfloat16
    AF = mybir.ActivationFunctionType
    OP = mybir.AluOpType

    def S_of(g, p):
        return slot_sizes[p * NG + g]

    def t_of(g, p):
        return g * GP + p

    # column offsets within each group's buffer
    qoff = [[0] * GP for _ in range(NG)]
    for g in range(NG):
        acc = 0
        for p in range(GP):
            qoff[g][p] = acc
            acc += S_of(g, p)
        assert acc <= GR

    nc = bacc.Bacc("TRN2", target_bir_lowering=False)
    xt_d = nc.dram_tensor("xt", [NG, 128, NCH, GR], bf16, kind="ExternalInput")
    cnt_d = nc.dram_tensor("cnt", [128, 5, NCH, CLS], f32, kind="ExternalInput")
    out_d = nc.dram_tensor("outv", [1, NF], f32, kind="ExternalOutput")

    V = nc.vector
    A = nc.scalar
    P = nc.gpsimd
    T = nc.tensor

    with tile.TileContext(nc) as tc:
        with (
            tc.tile_pool(name="persist", bufs=1) as persist,
            tc.tile_pool(name="stats", bufs=1) as stats,
            tc.tile_pool(name="sqscr", bufs=2) as sqscr_pool,
            tc.tile_pool(name="gram", bufs=4, space="PSUM") as gram,
            tc.tile_pool(name="fpsum", bufs=1, space="PSUM") as fpsum,
        ):
            x_g = [
                persist.tile([128, NCH, GR], bf16, tag=f"x{g}", name=f"x{g}")
                for g in range(NG)
            ]
            z_g = [
                persist.tile([128, NCH, GR], bf16, tag=f"z{g}", name=f"z{g}")
                for g in range(NG)
            ]
            cnt_sb = persist.tile([128, 5, NCH, CLS], f32, tag="cnt")
            bnbuf = persist.tile([128, NCH, CLS, 6], f32, tag="bnbuf")
            fin = persist.tile([128, NF], f32, tag="fin")
            ones = persist.tile([128, 1], f32, tag="ones")
            zeros_bf = persist.tile([128, 512], bf16, tag="zbf")

            # ---- init (Pool engine; keep DVE clean) ----
            P.memset(fin, 0.0)
            P.memset(ones, 1.0)
            P.memset(zeros_bf, 0.0)

            # ---- DMA in ----
            nc.sync.dma_start(out=cnt_sb, in_=cnt_d[:, :, :, :])
            for g in range(NG):
                nc.sync.dma_start(out=x_g[g], in_=xt_d[g, :, :, :])

            # ---- stats tiles ----
            def st(tag):
                return stats.tile([128, NCH, CLS], f32, tag=tag, name=tag)

            t1 = st("t1")
            t2 = st("t2")
            s1 = st("s1")
            s2 = st("s2")
            mu = st("mu")
            m2 = st("m2")
            tt = st("tt")
            var = st("var")
            tv = st("tv")
            sq = st("sq")
            r = st("r")
            mur = st("mur")
            nmur = st("nmur")
            r2 = st("r2")
            d = st("d")
            if NEWTON:
                r0 = st("r0")
                w = st("w")

            me = bnbuf[:, :, :, 1]
            ve = bnbuf[:, :, :, 2]
            mo = bnbuf[:, :, :, 4]
            vo = bnbuf[:, :, :, 5]
            Svec = cnt_sb[:, 0, :, :]
            invS = cnt_sb[:, 1, :, :]
            in1v = cnt_sb[:, 2, :, :]
            cev = cnt_sb[:, 3, :, :]
            cov = cnt_sb[:, 4, :, :]

            dsq_view = fin[:, DSQ0 : DSQ0 + 64].rearrange("p (c k) -> p c k", c=NCH)

            # ---- emission helpers ----
            def emit_bn(g):
                for p in range(GP):
                    t = t_of(g, p)
                    S = S_of(g, p)
                    q = qoff[g][p]
                    for ch in range(NCH):
                        V.bn_stats(
                            out=bnbuf[:, ch, t, :],
                            in_=x_g[g][:, ch, q : q + S],
                        )

            def emit_stats_pool(g):
                gs = slice(t_of(g, 0), t_of(g, 0) + GP)

                def sl(ap):
                    return ap[:, :, gs]

                TT = P.tensor_tensor
                TT(out=sl(t1), in0=sl(me), in1=sl(cev), op=OP.mult)
                TT(out=sl(t2), in0=sl(mo), in1=sl(cov), op=OP.mult)
                TT(out=sl(s1), in0=sl(t1), in1=sl(t2), op=OP.add)
                TT(out=sl(s2), in0=sl(ve), in1=sl(vo), op=OP.add)
                TT(out=sl(t1), in0=sl(t1), in1=sl(me), op=OP.mult)  # e2
                TT(out=sl(t2), in0=sl(t2), in1=sl(mo), op=OP.mult)  # o2
                TT(out=sl(s2), in0=sl(s2), in1=sl(t1), op=OP.add)
                TT(out=sl(s2), in0=sl(s2), in1=sl(t2), op=OP.add)
                TT(out=sl(mu), in0=sl(s1), in1=sl(invS), op=OP.mult)
                TT(out=sl(m2), in0=sl(mu), in1=sl(mu), op=OP.mult)
                TT(out=sl(m2), in0=sl(m2), in1=sl(Svec), op=OP.mult)
                TT(out=sl(tt), in0=sl(s2), in1=sl(m2), op=OP.subtract)
                TT(out=sl(var), in0=sl(tt), in1=sl(in1v), op=OP.mult)
                P.tensor_scalar_max(out=sl(var), in0=sl(var), scalar1=0.0)
                P.tensor_scalar_add(out=sl(tv), in0=sl(var), scalar1=float(EPS))

            def emit_sqrt(g):
                gs = slice(t_of(g, 0), t_of(g, 0) + GP)
                A.sqrt(out=sq[:, :, gs], in_=tv[:, :, gs])

            def emit_recip(g):
                gs = slice(t_of(g, 0), t_of(g, 0) + GP)
                if NEWTON:
                    V.reciprocal(out=r0[:, :, gs], in_=sq[:, :, gs])
                else:
                    V.reciprocal(out=r[:, :, gs], in_=sq[:, :, gs])

            def emit_newton_pool(g):
                if not NEWTON:
                    return
                gs = slice(t_of(g, 0), t_of(g, 0) + GP)

                def sl(ap):
                    return ap[:, :, gs]

                P.tensor_tensor(out=sl(w), in0=sl(r0), in1=sl(r0), op=OP.mult)
                P.tensor_tensor(out=sl(w), in0=sl(tv), in1=sl(w), op=OP.mult)
                P.tensor_scalar(
                    out=sl(w), in0=sl(w), scalar1=-0.5, scalar2=1.5,
                    op0=OP.mult, op1=OP.add,
                )
                P.tensor_tensor(out=sl(r), in0=sl(r0), in1=sl(w), op=OP.mult)

            def emit_mur_pool(g):
                gs = slice(t_of(g, 0), t_of(g, 0) + GP)
                P.tensor_tensor(
                    out=mur[:, :, gs], in0=mu[:, :, gs], in1=r[:, :, gs],
                    op=OP.mult,
                )
                P.tensor_scalar_mul(
                    out=nmur[:, :, gs], in0=mur[:, :, gs], scalar1=-1.0
                )

            def emit_norm(g):
                eng = NORM_ENG[g]
                for p in range(GP):
                    t = t_of(g, p)
                    S = S_of(g, p)
                    q = qoff[g][p]
                    for ch in range(NCH):
                        zsl = z_g[g][:, ch, q : q + S]
                        xsl = x_g[g][:, ch, q : q + S]
                        if eng == "act":
                            A.activation(
                                out=zsl,
                                in_=xsl,
                                func=AF.Identity,
                                scale=r[:, ch, t : t + 1],
                                bias=nmur[:, ch, t : t + 1],
                            )
                        elif eng == "dve":
                            V.tensor_scalar(
                                out=zsl,
                                in0=xsl,
                                scalar1=mu[:, ch, t : t + 1],
                                scalar2=r[:, ch, t : t + 1],
                                op0=OP.subtract,
                                op1=OP.mult,
                            )
                        else:
                            P.tensor_scalar(
                                out=zsl,
                                in0=xsl,
                                scalar1=mu[:, ch, t : t + 1],
                                scalar2=r[:, ch, t : t + 1],
                                op0=OP.subtract,
                                op1=OP.mult,
                            )

            ps_tiles = {}

            def emit_gram(g, p):
                t = t_of(g, p)
                S = S_of(g, p)
                q = qoff[g][p]
                ps = gram.tile([128, 2, 256], f32, tag="ps", name=f"ps{g}_{p}")
                ps_tiles[(g, p)] = ps
                if g == 0:
                    # zero the whole bank once (start of its ascending chain)
                    T.matmul(
                        ps[:, :, :],
                        lhsT=zeros_bf[:, 0:128],
                        rhs=zeros_bf[:, 0:512],
                        start=True,
                        stop=True,
                    )
                # rc0: rows 0:min(128,S)
                m0 = min(128, S)
                for ch in range(NCH):
                    T.matmul(
                        ps[0:m0, 0, 0:S],
                        lhsT=z_g[g][:, ch, q : q + m0],
                        rhs=z_g[g][:, ch, q : q + S],
                        start=(ch == 0),
                        stop=(ch == NCH - 1),
                    )
                if S > 128:
                    m1 = S - 128
                    for ch in range(NCH):
                        T.matmul(
                            ps[0:m1, 1, 0:S],
                            lhsT=z_g[g][:, ch, q + 128 : q + S],
                            rhs=z_g[g][:, ch, q : q + S],
                            start=(ch == 0),
                            stop=(ch == NCH - 1),
                        )

            def emit_square(g, p):
                t = t_of(g, p)
                S = S_of(g, p)
                ps = ps_tiles[(g, p)]
                scr = sqscr_pool.tile(
                    [128, 2, 256], bf16, tag="sqs", name=f"sqs{g}_{p}"
                )
                if SQ_ENG[g] == "split":
                    # ACT squares into SBUF bf16; DVE does the reduction
                    A.activation(
                        out=scr[:, :, 0:S],
                        in_=ps[:, :, 0:S],
                        func=AF.Square,
                    )
                    V.tensor_reduce(
                        out=fin[:, GSQ0 + 2 * t : GSQ0 + 2 * t + 2],
                        in_=scr[:, :, 0:S],
                        axis=mybir.AxisListType.X,
                        op=OP.add,
                    )
                else:
                    A.activation(
                        out=scr[:, :, 0:S],
                        in_=ps[:, :, 0:S],
                        func=AF.Square,
                        accum_out=fin[:, GSQ0 + 2 * t : GSQ0 + 2 * t + 1],
                    )

            # =========== emission schedule ===========
            # DVE: bn0 bn1 rec0 nmur0 n0d bn2 rec1 nmur1 n1d bn3 rec2 n2d
            #      rec3 n3d
            # ACT: sqrt0 n0a sqrt1 n1a sq0 sqrt2 sq1 sqrt3 sq2 sq3
            # Pool: stats0..3 dsq
            def dve_ps(g):
                return NORM_DVE_SLOTS[g]

            def act_ps(g):
                return tuple(p for p in range(GP) if p not in NORM_DVE_SLOTS[g])

            emit_bn(0)
            emit_bn(1)
            emit_stats_pool(0)
            emit_sqrt(0)
            emit_recip(0)
            if act_ps(0):
                emit_nmur_dve(0)
            emit_stats_pool(1)
            emit_norm(0, "dve", ps=dve_ps(0))
            emit_norm(0, "act", ps=act_ps(0))
            for p in range(GP):
                emit_gram(0, p)
            emit_bn(2)
            emit_sqrt(1)
            emit_recip(1)
            if act_ps(1):
                emit_nmur_dve(1)
            emit_stats_pool(2)
            emit_norm(1, "dve", ps=dve_ps(1))
            emit_norm(1, "act", ps=act_ps(1))
            for p in range(GP):
                emit_gram(1, p)
            emit_bn(3)
            emit_sqrt(2)
            emit_recip(2)
            if act_ps(2):
                emit_nmur_dve(2)
            emit_stats_pool(3)
            emit_norm(2, "dve", ps=dve_ps(2))
            emit_norm(2, "act", ps=act_ps(2))
            for p in range(GP):
                emit_gram(2, p)
                emit_square(0, p)
            emit_sqrt(3)
            emit_recip(3)
            if act_ps(3):
                emit_nmur_dve(3)
            emit_norm(3, "dve", ps=dve_ps(3))
            emit_norm(3, "act", ps=act_ps(3))
            for p in range(GP):
                emit_square(1, p)
            for p in range(GP):
                emit_gram(3, p)
                emit_square(2, p)
            for p in range(GP):
                emit_square(3, p)

            # lazy diag chain (Pool): d = tt*r^2 ; dsq = d*d -> fin
            P.tensor_tensor(out=r2, in0=r, in1=r, op=OP.mult)
            P.tensor_tensor(out=d, in0=tt, in1=r2, op=OP.mult)
            P.tensor_tensor(out=dsq_view, in0=d, in1=d, op=OP.mult)

            # ---- final partition reduction via ones-matmul ----
            fps = fpsum.tile([1, NF], f32, tag="fps")
            T.matmul(fps, lhsT=ones, rhs=fin, start=True, stop=True)
            outsb = persist.tile([1, NF], f32, tag="outsb")
            V.tensor_copy(out=outsb, in_=fps)
            nc.sync.dma_start(out=out_d[:, :], in_=outsb)

    nc.compile()
    return nc


def _ensure_axon_ntff_hook():
    """Register the axon NTFF profiling hook if the image's antenv lacks it."""
    try:
        import types

        import antenv

        try:
            from antenv.axon_hooks import get_axon_ntff_profile_hook  # noqa: F401

            return
        except ImportError:
            pass
        from trn_agent_boot.trn_boot import _ntff_profile_via_ctypes

        mod = types.ModuleType("antenv.axon_hooks")
        _st = {"hook": None}
        mod.set_axon_ntff_profile_hook = lambda h: _st.update(hook=h)
        mod.get_axon_ntff_profile_hook = lambda: _st["hook"]
        sys.modules["antenv.axon_hooks"] = mod
        antenv.axon_hooks = mod
        mod.set_axon_ntff_profile_hook(
            _ntff_profile_via_ctypes("/opt/axon/libaxon_pjrt.so")
        )
        import concourse.bass_utils as _bu

        _bu.upload_artifacts = lambda tmpdir: tmpdir
    except Exception as e:  # profiling is best-effort
        print(f"ntff hook registration failed: {e}", file=sys.stderr)


def _shard(y: np.ndarray):
    counts = np.bincount(y, minlength=K).astype(np.int64)
    order = np.argsort(-counts, kind="stable")
    core_classes = [[] for _ in range(NCORES)]
    for i, cls in enumerate(order):
        row, col = i // NCORES, i % NCORES
        core = col if row % 2 == 0 else NCORES - 1 - col
        core_classes[core].append(int(cls))
    # sort each core's classes ascending by count -> rank r
    for c in range(NCORES):
        core_classes[c].sort(key=lambda k: counts[k])
    slot_sizes = [0] * CLS
    for rank in range(CLS):
        m = max(int(counts[core_classes[c][rank]]) for c in range(NCORES))
        S = max(m, 2)
        S = (S + 1) // 2 * 2  # even, for 4B-aligned bf16 slices
        assert S <= 256, "class too large for psum bank layout"
        slot_sizes[rank] = S
    assert all(
        slot_sizes[rank] <= slot_sizes[rank + 1] for rank in range(CLS - 1)
    )
    return counts, core_classes, tuple(slot_sizes)


def kernel(x: np.ndarray, y: np.ndarray) -> np.ndarray:
    x = np.ascontiguousarray(np.asarray(x, dtype=np.float32))
    y = np.asarray(y).astype(np.int64).ravel()
    N = x.shape[0]
    assert x.shape == (N, C)

    counts, core_classes, slot_sizes = _shard(y)

    def S_of(g, p):
        return slot_sizes[p * NG + g]

    GR = max(sum(S_of(g, p) for p in range(GP)) for g in range(NG))
    GR = (GR + 7) // 8 * 8
    qoff = [[0] * GP for _ in range(NG)]
    for g in range(NG):
        acc = 0
        for p in range(GP):
            qoff[g][p] = acc
            acc += S_of(g, p)

    key = (GR, slot_sizes)
    if key not in _nc_cache:
        _nc_cache[key] = _build_nc(slot_sizes, GR)
    nc = _nc_cache[key]

    # ---- build per-core inputs ----
    xTfull = np.ascontiguousarray(x.T)  # [C, N]
    in_maps = []
    for j in range(NCORES):
        xt = np.zeros((NG, 128, NCH, GR), dtype=np.float32)
        cnt = np.zeros((128, 5, NCH, CLS), dtype=np.float32)
        for rank in range(CLS):
            cls = core_classes[j][rank]
            g, p = rank % NG, rank // NG
            t = g * GP + p
            S = slot_sizes[rank]
            q = qoff[g][p]
            idx = np.flatnonzero(y == cls)
            n = len(idx)
            if n:
                blk = xTfull[:, idx].reshape(NCH, 128, n).transpose(1, 0, 2)
                xt[g, :, :, q : q + n] = blk
                if n < S:
                    muf = xTfull[:, idx].mean(axis=1)  # [C]
                    mu128 = muf.reshape(NCH, 128).T  # [128, NCH]
                    xt[g, :, :, q + n : q + S] = mu128[:, :, None]
            ce, co = (S + 1) // 2, S // 2
            cnt[:, 0, :, t] = ce / S
            cnt[:, 1, :, t] = co / S
            cnt[:, 2, :, t] = ce * co / S
            cnt[:, 3, :, t] = 1.0 / max(n - 1, 1)
        in_maps.append({"xt": xt.astype(BF16), "cnt": cnt})

    trace = bool(int(os.environ.get("KERNEL_TRACE", "0")))
    if trace:
        _ensure_axon_ntff_hook()
    res = run_bass_kernel_spmd(
        nc,
        in_maps,
        core_ids=list(range(NCORES)),
        trace=trace,
        **({"trace_cores": [0], "stitch_traces": False} if trace else {}),
    )
    global _last_results
    _last_results = res

    # ---- host combine ----
    off_denom = np.float64(C * (C - 1))
    loss_num = np.float64(0.0)
    n_count = np.float64(0.0)
    for j in range(NCORES):
        o = np.asarray(res.results[j]["outv"], dtype=np.float64).reshape(NF)
        for rank in range(CLS):
            cls = core_classes[j][rank]
            n = int(counts[cls])
            if n <= 1:
                continue
            g, p = rank % NG, rank // NG
            t = g * GP + p
            gsq = o[GSQ0 + t]
            dsum = sum(o[DSQ0 + ch * CLS + t] for ch in range(NCH))
            off_sum = gsq - dsum
            loss_num += off_sum / off_denom
            n_count += n
    out = loss_num / n_count if n_count > 0 else 0.0
    return np.float32(out)


# revision 19
# speedup vs baseline: 1.5592x; 1.0023x over previous
"""Trainium2 Bass kernel for nn_DecorrelateLossClass (segment_reduce / ridge).

Class-sharded, collective-free, bf16 data path:
  * 128 classes -> 16 per core (snake by descending count); within a core
    classes sort ASCENDING into slot ranks r (r = p*4+g), giving 4 balanced
    pipeline groups and ascending per-PSUM-bank widths (stale-region
    safety for the Frobenius reads).
  * Host packs x feature-major bf16 per group; class slots are padded with
    the CLASS MEAN so normalized pads are ~0 (no phantom corrections).
  * Device pipeline per group: bn_stats (DVE, even/odd fields give padded
    sums) -> Welford-merge stats math (Pool) -> sqrt (ACT) + reciprocal
    (DVE) -> z=(x-mu)*r in bf16 (ACT for early groups, DVE 4x-mode for
    late ones) -> per-class exact Gram Z_k^T Z_k (PE, bf16) -> Frobenius
    via ACT Square+accum (PSUM pads are guaranteed zero by a PE
    zero-matmul + ascending bank schedule).
  * diag(corr)^2 analytic from stats (Pool); ones-matmul partition
    reduce; host sums 8x16 per-class scalars.
  * Every stats tile is split per group to avoid whole-tile WAR/WAW
    false serialization; a dummy sqrt preloads the ACT table; PE warmup
    matmuls ramp the clock during the DMA prologue.
"""

import os
import sys

import numpy as np

for _p in ("/opt/trn_rl_repo",):
    if os.path.isdir(_p) and _p not in sys.path:
        sys.path.insert(0, _p)

import concourse.bass as bass
from concourse import bacc
import concourse.mybir as mybir
import concourse.tile as tile
from concourse.bass_utils import run_bass_kernel_spmd

import ml_dtypes

BF16 = ml_dtypes.bfloat16

K = 128
C = 512
NCH = 4  # feature chunks of 128
NCORES = 8
CLS = 16  # classes per core
NG = 4  # pipeline groups
GP = CLS // NG  # slots per group
EPS = 1e-8

# fin layout: gsq per stats-index t at cols [2t, 2t+1], dsq [32:96] (ch*16+t)
GSQ0, DSQ0, NF = 0, 16, 96

# engine assignment knobs (tuned from traces)
# norm engine per group: ACT handles early groups (DVE busy with bn_stats),
# DVE takes the late ones once bn is drained.
# slots (p indices) normalized on DVE per group; the rest go to ACT
NORM_DVE_SLOTS = {0: (), 1: (), 2: (0, 1, 2, 3), 3: (0, 1, 2, 3)}
N_WARMUP_MM = 24  # junk matmuls to ramp the PE clock before real grams
NEWTON = False

_nc_cache: dict = {}
_last_results = None


def _build_nc(slot_sizes: tuple, GR: int):
    """slot_sizes indexed by rank r (ascending sizes); r = p*NG + g."""
    f32 = mybir.dt.float32
    bf16 = mybir.dt.bfloat16
    AF = mybir.ActivationFunctionType
    OP = mybir.AluOpType

    def S_of(g, p):
        return slot_sizes[p * NG + g]

    def t_of(g, p):
        return g * GP + p

    # column offsets within each group's buffer
    qoff = [[0] * GP for _ in range(NG)]
    for g in range(NG):
        acc = 0
        for p in range(GP):
            qoff[g][p] = acc
            acc += S_of(g, p)
        assert acc <= GR

    nc = bacc.Bacc("TRN2", target_bir_lowering=False)
    xt_d = nc.dram_tensor("xt", [NG, 128, NCH, GR], bf16, kind="ExternalInput")
    cnt_d = nc.dram_tensor("cnt", [128, 5, NCH, CLS], f32, kind="ExternalInput")
    out_d = nc.dram_tensor("outv", [1, NF], f32, kind="ExternalOutput")

    V = nc.vector
    A = nc.scalar
    P = nc.gpsimd
    T = nc.tensor

    with tile.TileContext(nc) as tc:
        with (
            tc.tile_pool(name="persist", bufs=1) as persist,
            tc.tile_pool(name="stats", bufs=1) as stats,
            tc.tile_pool(name="sqscr", bufs=2) as sqscr_pool,
            tc.tile_pool(name="gram", bufs=6, space="PSUM") as gram,
            tc.tile_pool(name="fpsum", bufs=1, space="PSUM") as fpsum,
        ):
            x_g = [
                persist.tile([128, NCH, GR], bf16, tag=f"x{g}", name=f"x{g}")
                for g in range(NG)
            ]
            z_g = [
                persist.tile([128, NCH, GR], bf16, tag=f"z{g}", name=f"z{g}")
                for g in range(NG)
            ]
            cnt_sb = persist.tile([128, 5, NCH, CLS], f32, tag="cnt")
            bnbuf = persist.tile([128, NCH, CLS, 6], f32, tag="bnbuf")
            fin = persist.tile([128, NF], f32, tag="fin")
            ones = persist.tile([128, 1], f32, tag="ones")
            zeros_bf = persist.tile([128, 512], bf16, tag="zbf")

            # ---- init (Pool engine; keep DVE clean) ----
            P.memset(fin, 0.0)
            P.memset(ones, 1.0)
            P.memset(zeros_bf, 0.0)

            # ---- DMA in ----
            nc.sync.dma_start(out=cnt_sb, in_=cnt_d[:, :, :, :])
            for g in range(NG):
                nc.sync.dma_start(out=x_g[g], in_=xt_d[g, :, :, :])

            # ---- stats tiles ----
            def st(tag):
                return stats.tile([128, NCH, CLS], f32, tag=tag, name=tag)

            t1 = st("t1")
            t2 = st("t2")
            s1 = st("s1")
            s2 = st("s2")
            mu = st("mu")
            m2 = st("m2")
            tt = st("tt")
            var = st("var")
            tv = st("tv")
            sq = st("sq")
            r = st("r")
            mur = st("mur")
            nmur = st("nmur")
            r2 = st("r2")
            d = st("d")
            if NEWTON:
                r0 = st("r0")
                w = st("w")

            me = bnbuf[:, :, :, 1]
            ve = bnbuf[:, :, :, 2]
            mo = bnbuf[:, :, :, 4]
            vo = bnbuf[:, :, :, 5]
            Svec = cnt_sb[:, 0, :, :]
            invS = cnt_sb[:, 1, :, :]
            in1v = cnt_sb[:, 2, :, :]
            cev = cnt_sb[:, 3, :, :]
            cov = cnt_sb[:, 4, :, :]

            dsq_view = fin[:, DSQ0 : DSQ0 + 64].rearrange("p (c k) -> p c k", c=NCH)

            # ---- emission helpers ----
            def emit_bn(g):
                for p in range(GP):
                    t = t_of(g, p)
                    S = S_of(g, p)
                    q = qoff[g][p]
                    for ch in range(NCH):
                        V.bn_stats(
                            out=bnbuf[:, ch, t, :],
                            in_=x_g[g][:, ch, q : q + S],
                        )

            def emit_stats_pool(g):
                gs = slice(t_of(g, 0), t_of(g, 0) + GP)

                def sl(ap):
                    return ap[:, :, gs]

                TT = P.tensor_tensor
                TT(out=sl(t1), in0=sl(me), in1=sl(cev), op=OP.mult)
                TT(out=sl(t2), in0=sl(mo), in1=sl(cov), op=OP.mult)
                TT(out=sl(s1), in0=sl(t1), in1=sl(t2), op=OP.add)
                TT(out=sl(s2), in0=sl(ve), in1=sl(vo), op=OP.add)
                TT(out=sl(t1), in0=sl(t1), in1=sl(me), op=OP.mult)  # e2
                TT(out=sl(t2), in0=sl(t2), in1=sl(mo), op=OP.mult)  # o2
                TT(out=sl(s2), in0=sl(s2), in1=sl(t1), op=OP.add)
                TT(out=sl(s2), in0=sl(s2), in1=sl(t2), op=OP.add)
                TT(out=sl(mu), in0=sl(s1), in1=sl(invS), op=OP.mult)
                TT(out=sl(m2), in0=sl(mu), in1=sl(mu), op=OP.mult)
                TT(out=sl(m2), in0=sl(m2), in1=sl(Svec), op=OP.mult)
                TT(out=sl(tt), in0=sl(s2), in1=sl(m2), op=OP.subtract)
                TT(out=sl(var), in0=sl(tt), in1=sl(in1v), op=OP.mult)
                P.tensor_scalar_max(out=sl(var), in0=sl(var), scalar1=0.0)
                P.tensor_scalar_add(out=sl(tv), in0=sl(var), scalar1=float(EPS))

            def emit_sqrt(g):
                gs = slice(t_of(g, 0), t_of(g, 0) + GP)
                A.sqrt(out=sq[:, :, gs], in_=tv[:, :, gs])

            def emit_recip(g):
                gs = slice(t_of(g, 0), t_of(g, 0) + GP)
                if NEWTON:
                    V.reciprocal(out=r0[:, :, gs], in_=sq[:, :, gs])
                else:
                    V.reciprocal(out=r[:, :, gs], in_=sq[:, :, gs])

            def emit_newton_pool(g):
                if not NEWTON:
                    return
                gs = slice(t_of(g, 0), t_of(g, 0) + GP)

                def sl(ap):
                    return ap[:, :, gs]

                P.tensor_tensor(out=sl(w), in0=sl(r0), in1=sl(r0), op=OP.mult)
                P.tensor_tensor(out=sl(w), in0=sl(tv), in1=sl(w), op=OP.mult)
                P.tensor_scalar(
                    out=sl(w), in0=sl(w), scalar1=-0.5, scalar2=1.5,
                    op0=OP.mult, op1=OP.add,
                )
                P.tensor_tensor(out=sl(r), in0=sl(r0), in1=sl(w), op=OP.mult)

            def emit_mur_pool(g):
                gs = slice(t_of(g, 0), t_of(g, 0) + GP)
                P.tensor_tensor(
                    out=mur[:, :, gs], in0=mu[:, :, gs], in1=r[:, :, gs],
                    op=OP.mult,
                )
                P.tensor_scalar_mul(
                    out=nmur[:, :, gs], in0=mur[:, :, gs], scalar1=-1.0
                )

            def emit_norm(g):
                eng = NORM_ENG[g]
                for p in range(GP):
                    t = t_of(g, p)
                    S = S_of(g, p)
                    q = qoff[g][p]
                    for ch in range(NCH):
                        zsl = z_g[g][:, ch, q : q + S]
                        xsl = x_g[g][:, ch, q : q + S]
                        if eng == "act":
                            A.activation(
                                out=zsl,
                                in_=xsl,
                                func=AF.Identity,
                                scale=r[:, ch, t : t + 1],
                                bias=nmur[:, ch, t : t + 1],
                            )
                        elif eng == "dve":
                            V.tensor_scalar(
                                out=zsl,
                                in0=xsl,
                                scalar1=mu[:, ch, t : t + 1],
                                scalar2=r[:, ch, t : t + 1],
                                op0=OP.subtract,
                                op1=OP.mult,
                            )
                        else:
                            P.tensor_scalar(
                                out=zsl,
                                in0=xsl,
                                scalar1=mu[:, ch, t : t + 1],
                                scalar2=r[:, ch, t : t + 1],
                                op0=OP.subtract,
                                op1=OP.mult,
                            )

            ps_tiles = {}

            def emit_gram(g, p):
                t = t_of(g, p)
                S = S_of(g, p)
                q = qoff[g][p]
                ps = gram.tile([128, 2, 256], f32, tag="ps", name=f"ps{g}_{p}")
                ps_tiles[(g, p)] = ps
                # zero the bank where the 6-deep rotation's occupant-width
                # sequence starts or descends (stale-region safety)
                ZSET = {(0, 0), (0, 1), (0, 2), (0, 3), (1, 0), (1, 1),
                        (2, 0), (2, 1), (3, 0), (3, 1)}
                if (g, p) in ZSET:
                    # zero the whole bank once (start of its ascending chain)
                    T.matmul(
                        ps[:, :, :],
                        lhsT=zeros_bf[:, 0:128],
                        rhs=zeros_bf[:, 0:512],
                        start=True,
                        stop=True,
                    )
                # rc0: rows 0:min(128,S)
                m0 = min(128, S)
                for ch in range(NCH):
                    T.matmul(
                        ps[0:m0, 0, 0:S],
                        lhsT=z_g[g][:, ch, q : q + m0],
                        rhs=z_g[g][:, ch, q : q + S],
                        start=(ch == 0),
                        stop=(ch == NCH - 1),
                    )
                if S > 128:
                    m1 = S - 128
                    for ch in range(NCH):
                        T.matmul(
                            ps[0:m1, 1, 0:S],
                            lhsT=z_g[g][:, ch, q + 128 : q + S],
                            rhs=z_g[g][:, ch, q : q + S],
                            start=(ch == 0),
                            stop=(ch == NCH - 1),
                        )

            def emit_square(g, p):
                t = t_of(g, p)
                S = S_of(g, p)
                ps = ps_tiles[(g, p)]
                scr = sqscr_pool.tile(
                    [128, 2, 256], bf16, tag="sqs", name=f"sqs{g}_{p}"
                )
                if SQ_ENG[g] == "split":
                    # ACT squares into SBUF bf16; DVE does the reduction
                    A.activation(
                        out=scr[:, :, 0:S],
                        in_=ps[:, :, 0:S],
                        func=AF.Square,
                    )
                    V.tensor_reduce(
                        out=fin[:, GSQ0 + 2 * t : GSQ0 + 2 * t + 2],
                        in_=scr[:, :, 0:S],
                        axis=mybir.AxisListType.X,
                        op=OP.add,
                    )
                else:
                    A.activation(
                        out=scr[:, :, 0:S],
                        in_=ps[:, :, 0:S],
                        func=AF.Square,
                        accum_out=fin[:, GSQ0 + 2 * t : GSQ0 + 2 * t + 1],
                    )

            # =========== emission schedule ===========
            # DVE: bn0 bn1 rec0 nmur0 n0d bn2 rec1 nmur1 n1d bn3 rec2 n2d
            #      rec3 n3d
            # ACT: sqrt0 n0a sqrt1 n1a sq0 sqrt2 sq1 sqrt3 sq2 sq3
            # Pool: stats0..3 dsq
            def dve_ps(g):
                return NORM_DVE_SLOTS[g]

            def act_ps(g):
                return tuple(p for p in range(GP) if p not in NORM_DVE_SLOTS[g])

            emit_bn(0)
            emit_bn(1)
            emit_stats_pool(0)
            emit_sqrt(0)
            emit_recip(0)
            if act_ps(0):
                emit_nmur_dve(0)
            emit_stats_pool(1)
            emit_norm(0, "dve", ps=dve_ps(0))
            emit_norm(0, "act", ps=act_ps(0))
            for p in range(GP):
                emit_gram(0, p)
            emit_bn(2)
            emit_sqrt(1)
            emit_recip(1)
            if act_ps(1):
                emit_nmur_dve(1)
            emit_stats_pool(2)
            emit_norm(1, "dve", ps=dve_ps(1))
            emit_norm(1, "act", ps=act_ps(1))
            for p in range(GP):
                emit_gram(1, p)
            emit_bn(3)
            emit_sqrt(2)
            emit_recip(2)
            if act_ps(2):
                emit_nmur_dve(2)
            emit_stats_pool(3)
            emit_norm(2, "dve", ps=dve_ps(2))
            emit_norm(2, "act", ps=act_ps(2))
            for p in range(GP):
                emit_gram(2, p)
                emit_square(0, p)
            emit_sqrt(3)
            emit_recip(3)
            if act_ps(3):
                emit_nmur_dve(3)
            emit_norm(3, "dve", ps=dve_ps(3))
            emit_norm(3, "act", ps=act_ps(3))
            for p in range(GP):
                emit_square(1, p)
            for p in range(GP):
                emit_gram(3, p)
                emit_square(2, p)
            for p in range(GP):
                emit_square(3, p)

            # lazy diag chain (Pool): d = tt*r^2 ; dsq = d*d -> fin
            P.tensor_tensor(out=r2, in0=r, in1=r, op=OP.mult)
            P.tensor_tensor(out=d, in0=tt, in1=r2, op=OP.mult)
            P.tensor_tensor(out=dsq_view, in0=d, in1=d, op=OP.mult)

            # ---- final partition reduction via ones-matmul ----
            fps = fpsum.tile([1, NF], f32, tag="fps")
            T.matmul(fps, lhsT=ones, rhs=fin, start=True, stop=True)
            outsb = persist.tile([1, NF], f32, tag="outsb")
            V.tensor_copy(out=outsb, in_=fps)
            nc.sync.dma_start(out=out_d[:, :], in_=outsb)

    nc.compile()
    return nc


def _ensure_axon_ntff_hook():
    """Register the axon NTFF profiling hook if the image's antenv lacks it."""
    try:
        import types

        import antenv

        try:
            from antenv.axon_hooks import get_axon_ntff_profile_hook  # noqa: F401

            return
        except ImportError:
            pass
        from trn_agent_boot.trn_boot import _ntff_profile_via_ctypes

        mod = types.ModuleType("antenv.axon_hooks")
        _st = {"hook": None}
        mod.set_axon_ntff_profile_hook = lambda h: _st.update(hook=h)
        mod.get_axon_ntff_profile_hook = lambda: _st["hook"]
        sys.modules["antenv.axon_hooks"] = mod
        antenv.axon_hooks = mod
        mod.set_axon_ntff_profile_hook(
            _ntff_profile_via_ctypes("/opt/axon/libaxon_pjrt.so")
        )
        import concourse.bass_utils as _bu

        _bu.upload_artifacts = lambda tmpdir: tmpdir
    except Exception as e:  # profiling is best-effort
        print(f"ntff hook registration failed: {e}", file=sys.stderr)


def _shard(y: np.ndarray):
    counts = np.bincount(y, minlength=K).astype(np.int64)
    order = np.argsort(-counts, kind="stable")
    core_classes = [[] for _ in range(NCORES)]
    for i, cls in enumerate(order):
        row, col = i // NCORES, i % NCORES
        core = col if row % 2 == 0 else NCORES - 1 - col
        core_classes[core].append(int(cls))
    # sort each core's classes ascending by count -> rank r
    for c in range(NCORES):
        core_classes[c].sort(key=lambda k: counts[k])
    slot_sizes = [0] * CLS
    for rank in range(CLS):
        m = max(int(counts[core_classes[c][rank]]) for c in range(NCORES))
        S = max(m, 2)
        S = (S + 1) // 2 * 2  # even, for 4B-aligned bf16 slices
        assert S <= 256, "class too large for psum bank layout"
        slot_sizes[rank] = S
    assert all(
        slot_sizes[rank] <= slot_sizes[rank + 1] for rank in range(CLS - 1)
    )
    return counts, core_classes, tuple(slot_sizes)


def kernel(x: np.ndarray, y: np.ndarray) -> np.ndarray:
    x = np.ascontiguousarray(np.asarray(x, dtype=np.float32))
    y = np.asarray(y).astype(np.int64).ravel()
    N = x.shape[0]
    assert x.shape == (N, C)

    counts, core_classes, slot_sizes = _shard(y)

    def S_of(g, p):
        return slot_sizes[p * NG + g]

    GR = max(sum(S_of(g, p) for p in range(GP)) for g in range(NG))
    GR = (GR + 7) // 8 * 8
    qoff = [[0] * GP for _ in range(NG)]
    for g in range(NG):
        acc = 0
        for p in range(GP):
            qoff[g][p] = acc
            acc += S_of(g, p)

    key = (GR, slot_sizes)
    if key not in _nc_cache:
        _nc_cache[key] = _build_nc(slot_sizes, GR)
    nc = _nc_cache[key]

    # ---- build per-core inputs ----
    xTfull = np.ascontiguousarray(x.T)  # [C, N]
    in_maps = []
    for j in range(NCORES):
        xt = np.zeros((NG, 128, NCH, GR), dtype=np.float32)
        cnt = np.zeros((128, 5, NCH, CLS), dtype=np.float32)
        for rank in range(CLS):
            cls = core_classes[j][rank]
            g, p = rank % NG, rank // NG
            t = g * GP + p
            S = slot_sizes[rank]
            q = qoff[g][p]
            idx = np.flatnonzero(y == cls)
            n = len(idx)
            if n:
                blk = xTfull[:, idx].reshape(NCH, 128, n).transpose(1, 0, 2)
                xt[g, :, :, q : q + n] = blk
                if n < S:
                    muf = xTfull[:, idx].mean(axis=1)  # [C]
                    mu128 = muf.reshape(NCH, 128).T  # [128, NCH]
                    xt[g, :, :, q + n : q + S] = mu128[:, :, None]
            ce, co = (S + 1) // 2, S // 2
            cnt[:, 0, :, t] = ce / S
            cnt[:, 1, :, t] = co / S
            cnt[:, 2, :, t] = ce * co / S
            cnt[:, 3, :, t] = 1.0 / max(n - 1, 1)
        in_maps.append({"xt": xt.astype(BF16), "cnt": cnt})

    trace = bool(int(os.environ.get("KERNEL_TRACE", "0")))
    if trace:
        _ensure_axon_ntff_hook()
    res = run_bass_kernel_spmd(
        nc,
        in_maps,
        core_ids=list(range(NCORES)),
        trace=trace,
        **({"trace_cores": [0], "stitch_traces": False} if trace else {}),
    )
    global _last_results
    _last_results = res

    # ---- host combine ----
    off_denom = np.float64(C * (C - 1))
    loss_num = np.float64(0.0)
    n_count = np.float64(0.0)
    for j in range(NCORES):
        o = np.asarray(res.results[j]["outv"], dtype=np.float64).reshape(NF)
        for rank in range(CLS):
            cls = core_classes[j][rank]
            n = int(counts[cls])
            if n <= 1:
                continue
            g, p = rank % NG, rank // NG
            t = g * GP + p
            gsq = o[GSQ0 + t]
            dsum = sum(o[DSQ0 + ch * CLS + t] for ch in range(NCH))
            off_sum = gsq - dsum
            loss_num += off_sum / off_denom
            n_count += n
    out = loss_num / n_count if n_count > 0 else 0.0
    return np.float32(out)
